# revision 18
# baseline (speedup 1.0000x reference)
"""Trainium2 Bass kernel for a 3-relation GCN (gnn_message_passing).

Strategy (8 NeuronCores, SPMD):
  - Node dim (20000 -> padded 20480) sharded 2560/core.
  - Layer GCN matmuls computed locally; per-relation [2560,256] fp16 shards
    AllGathered to a full node-feature table in DRAM.
  - Edge aggregation: edges partitioned by destination (host side), sorted,
    chunked into 128-edge groups per 64-dst window.  h rows are fetched with
    gpsimd dma_gather; a host-built one-hot scatter matrix S (carrying the
    symmetric GCN norms + bias edges) turns the scatter-add into PE matmuls
    accumulating in PSUM.
  - MLPs run from DMA-transposed (xbar) activations so the contraction dim
    lands on partitions.
  - Segment-mean pooling is a one-hot (1/cnt) matmul; the [192,768] fp32
    partials are AllReduced; the tiny final MLP + log_softmax is replicated.
All device math fp16 with fp32 PSUM accumulation; final stage fp32.
"""

import sys

import numpy as np

sys.path.insert(0, "/opt/trn_rl_repo")

import concourse.bacc as bacc  # noqa: E402
import concourse.bass as bass  # noqa: E402
import concourse.mybir as mybir  # noqa: E402
import concourse.tile as tile  # noqa: E402
from concourse import bass_utils  # noqa: E402

F16 = mybir.dt.float16
F32 = mybir.dt.float32
I16 = mybir.dt.int16
AF = mybir.ActivationFunctionType
ALU = mybir.AluOpType
AX = mybir.AxisListType

NCORES = 8


class Cfg:
    def __init__(self, n=20000, e=320000, f_in=5189, dim=256, norig=183, ncls=7,
                 npad=20480, fpad=5248, winw=64, nsup=10):
        self.N = n
        self.E = e
        self.F_IN = f_in
        self.DIM = dim
        self.NORIG = norig
        self.C = ncls
        self.NPAD = npad
        self.SH = npad // NCORES          # nodes per core
        self.NB = self.SH // 128          # 128-dst blocks per core
        self.WINW = winw                  # dst window width for scatter matmuls
        self.NWIN = self.SH // winw
        self.WPB = 128 // winw            # windows per block
        self.FPAD = fpad
        self.KC1 = fpad // 128
        self.NSUP = nsup                  # x stripes
        self.SUPW = self.SH // nsup
        self.BIAS_ROW = npad
        self.HROWS = npad + 1
        self.SEGP = 192                   # padded NORIG
        self.KCAT = 3 * dim // 128        # 6
        self.KD = dim // 128              # 2
        assert self.SH % 128 == 0 and self.SUPW % 128 == 0 and fpad % 128 == 0


DEFAULT_CFG = Cfg()


# --------------------------------------------------------------------------
# host-side preprocessing
# --------------------------------------------------------------------------

def _wrap_idx(idx, cfg):
    """dma_gather index layout: pos i -> partition i%16, col i//16; the 16-row
    block replicated across the 8 GPSIMD cores (128 partitions)."""
    tg8 = idx.shape[0] // 16
    w = idx.astype(np.int16).reshape(tg8, 16).T            # [16, tg*8]
    return np.ascontiguousarray(np.tile(w, (8, 1)))        # [128, tg*8]


def prep(inputs, cfg=DEFAULT_CFG):
    c_ = cfg
    f16 = np.float16
    x = np.asarray(inputs["x"], np.float32)
    w1 = np.asarray(inputs["w1"], np.float32)
    b1 = np.asarray(inputs["b1"], np.float32)
    w2 = np.asarray(inputs["w2"], np.float32)
    b2 = np.asarray(inputs["b2"], np.float32)
    eis = [np.asarray(inputs[f"edge_index_{k}"], np.int64) for k in (1, 2, 3)]
    idx1 = np.asarray(inputs["index_1"], np.int64)
    idx3 = np.asarray(inputs["index_3"], np.int64)

    # ---- per-relation edge prep: self loops, degrees, norms, dst partition
    Gw = np.zeros((3, c_.NWIN), np.int64)   # groups per (relative) window, max over cores
    core_edges = [[None] * 3 for _ in range(NCORES)]
    loop = np.arange(c_.N)
    for r in range(3):
        src = np.concatenate([eis[r][0], loop])
        dst = np.concatenate([eis[r][1], loop])
        deg = np.bincount(dst, minlength=c_.N).astype(np.float64)
        dinv = 1.0 / np.sqrt(deg)
        norm = (dinv[src] * dinv[dst]).astype(np.float32)
        for c in range(NCORES):
            lo = c * c_.SH
            m = (dst >= lo) & (dst < lo + c_.SH)
            es, ed, en = src[m], dst[m] - lo, norm[m]
            nreal = int(min(max(c_.N - lo, 0), c_.SH))     # real dst rows in shard
            es = np.concatenate([es, np.full(nreal, c_.BIAS_ROW, np.int64)])
            ed = np.concatenate([ed, np.arange(nreal)])
            en = np.concatenate([en, np.ones(nreal, np.float32)])
            o = np.argsort(ed, kind="stable")
            es, ed, en = es[o], ed[o], en[o]
            core_edges[c][r] = (es, ed, en)
            cnts = np.bincount(ed // c_.WINW, minlength=c_.NWIN)
            Gw[r] = np.maximum(Gw[r], (cnts + 127) // 128)
    Gw = np.maximum(Gw, 1)  # every window needs >=1 group so its PSUM is written
    TG = Gw.sum(axis=1)

    # ---- pooling one-hot (1/cnt) matrices, global counts
    cnt1 = np.bincount(idx1, minlength=c_.NORIG).astype(np.float64)
    cnt3 = np.bincount(idx3, minlength=c_.NORIG).astype(np.float64)
    r1 = (1.0 / np.maximum(cnt1, 1.0)).astype(np.float32)
    r3 = (1.0 / np.maximum(cnt3, 1.0)).astype(np.float32)
    P1 = np.zeros((NCORES, 128, c_.NB, c_.SEGP), f16)
    P3 = np.zeros((NCORES, 128, c_.NB, c_.SEGP), f16)
    nn = np.arange(c_.N)
    cc, ll = nn // c_.SH, nn % c_.SH
    P1[cc, ll % 128, ll // 128, idx1] = r1[idx1].astype(f16)
    P3[cc, ll % 128, ll // 128, idx3] = r3[idx3].astype(f16)

    # ---- shared (replicated) tensors
    w1h = np.zeros((3, c_.FPAD, c_.DIM), f16)
    w1h[:, : c_.F_IN] = w1.astype(f16)
    shared = {
        "w1h": w1h,
        "w2h": w2.astype(f16),
        "b1row": b1.astype(f16).reshape(1, -1),
        "b2row": b2.astype(f16).reshape(1, -1),
        "m1w1h": np.asarray(inputs["m1w1"], np.float32).astype(f16),
        "m1w2h": np.asarray(inputs["m1w2"], np.float32).astype(f16),
        "m2w1h": np.asarray(inputs["m2w1"], np.float32).astype(f16),
        "m2w2h": np.asarray(inputs["m2w2"], np.float32).astype(f16),
        "m1b1bc": np.tile(np.asarray(inputs["m1b1"], np.float32), (128, 1)),
        "m1b2bc": np.tile(np.asarray(inputs["m1b2"], np.float32), (128, 1)),
        "m2b1bc": np.tile(np.asarray(inputs["m2b1"], np.float32), (128, 1)),
        "m2b2bc": np.tile(np.asarray(inputs["m2b2"], np.float32), (128, 1)),
        "mw1f": np.asarray(inputs["mw1"], np.float32),
        "mw2f": np.asarray(inputs["mw2"], np.float32),
        "mb1bc": np.tile(np.asarray(inputs["mb1"], np.float32), (128, 1)),
        "mb2bc": np.tile(np.asarray(inputs["mb2"], np.float32), (128, 1)),
        "ident": np.eye(128, dtype=np.float32),
    }

    # ---- per-core tensors
    in_maps = []
    for c in range(NCORES):
        d = dict(shared)
        lo = c * c_.SH
        nreal = int(min(max(c_.N - lo, 0), c_.SH))
        xs = np.zeros((c_.SH, c_.FPAD), f16)
        xs[:nreal, : c_.F_IN] = x[lo : lo + nreal].astype(f16)
        xT = np.ascontiguousarray(
            xs.T.reshape(c_.FPAD, c_.NSUP, c_.SUPW).transpose(1, 0, 2)
        )
        d["xT"] = xT
        for r in range(3):
            es, ed, en = core_edges[c][r]
            win = ed // c_.WINW
            tg = int(TG[r])
            sp = np.full(tg * 128, c_.BIAS_ROW, np.int64)
            dl = np.zeros(tg * 128, np.int64)
            nm = np.zeros(tg * 128, np.float32)
            st = np.searchsorted(win, np.arange(c_.NWIN), side="left")
            en_ = np.searchsorted(win, np.arange(c_.NWIN), side="right")
            g0 = 0
            for w in range(c_.NWIN):
                cnt = int(en_[w] - st[w])
                sl = slice(g0 * 128, g0 * 128 + cnt)
                sp[sl] = es[st[w] : en_[w]]
                dl[sl] = ed[st[w] : en_[w]] - w * c_.WINW
                nm[sl] = en[st[w] : en_[w]]
                g0 += int(Gw[r][w])
            S = np.zeros((tg * 128, c_.WINW), f16)
            S[np.arange(tg * 128), dl] = nm.astype(f16)  # pad rows write 0.0
            d[f"S_{r}"] = np.ascontiguousarray(
                S.reshape(tg, 128, c_.WINW).transpose(1, 0, 2).reshape(128, tg * c_.WINW)
            )
            d[f"gidx_{r}"] = _wrap_idx(sp, c_)
        d["P1"] = np.ascontiguousarray(P1[c])
        d["P3"] = np.ascontiguousarray(P3[c])
        in_maps.append(d)

    meta = {"Gw": Gw.astype(int), "TG": [int(t) for t in TG]}
    return meta, in_maps


# --------------------------------------------------------------------------
# device program
# --------------------------------------------------------------------------

def build(meta, cfg=DEFAULT_CFG, enable_asserts=False):
    c_ = cfg
    Gw, TG = meta["Gw"], meta["TG"]
    nc = bacc.Bacc(
        "TRN2",
        target_bir_lowering=False,
        debug=False,
        enable_asserts=enable_asserts,
        num_devices=NCORES,
    )

    # ---------------- DRAM I/O
    xT_d = nc.dram_tensor("xT", [c_.NSUP, c_.FPAD, c_.SUPW], F16, kind="ExternalInput")
    w1_d = nc.dram_tensor("w1h", [3, c_.FPAD, c_.DIM], F16, kind="ExternalInput")
    w2_d = nc.dram_tensor("w2h", [3, c_.DIM, c_.DIM], F16, kind="ExternalInput")
    b1_d = nc.dram_tensor("b1row", [1, 3 * c_.DIM], F16, kind="ExternalInput")
    b2_d = nc.dram_tensor("b2row", [1, 3 * c_.DIM], F16, kind="ExternalInput")
    m1w1_d = nc.dram_tensor("m1w1h", [3 * c_.DIM, c_.DIM], F16, kind="ExternalInput")
    m1w2_d = nc.dram_tensor("m1w2h", [c_.DIM, c_.DIM], F16, kind="ExternalInput")
    m2w1_d = nc.dram_tensor("m2w1h", [3 * c_.DIM, c_.DIM], F16, kind="ExternalInput")
    m2w2_d = nc.dram_tensor("m2w2h", [c_.DIM, c_.DIM], F16, kind="ExternalInput")
    m1b1_d = nc.dram_tensor("m1b1bc", [128, c_.DIM], F32, kind="ExternalInput")
    m1b2_d = nc.dram_tensor("m1b2bc", [128, c_.DIM], F32, kind="ExternalInput")
    m2b1_d = nc.dram_tensor("m2b1bc", [128, c_.DIM], F32, kind="ExternalInput")
    m2b2_d = nc.dram_tensor("m2b2bc", [128, c_.DIM], F32, kind="ExternalInput")
    mw1_d = nc.dram_tensor("mw1f", [3 * c_.DIM, c_.DIM], F32, kind="ExternalInput")
    mw2_d = nc.dram_tensor("mw2f", [c_.DIM, c_.C], F32, kind="ExternalInput")
    mb1_d = nc.dram_tensor("mb1bc", [128, c_.DIM], F32, kind="ExternalInput")
    mb2_d = nc.dram_tensor("mb2bc", [128, c_.C], F32, kind="ExternalInput")
    ident_d = nc.dram_tensor("ident", [128, 128], F32, kind="ExternalInput")
    S_d = [
        nc.dram_tensor(f"S_{r}", [128, TG[r] * c_.WINW], F16, kind="ExternalInput")
        for r in range(3)
    ]
    gidx_d = [
        nc.dram_tensor(f"gidx_{r}", [128, TG[r] * 8], I16, kind="ExternalInput")
        for r in range(3)
    ]
    P1_d = nc.dram_tensor("P1", [128, c_.NB, c_.SEGP], F16, kind="ExternalInput")
    P3_d = nc.dram_tensor("P3", [128, c_.NB, c_.SEGP], F16, kind="ExternalInput")
    out_d = nc.dram_tensor("out", [c_.NORIG, c_.C], F32, kind="ExternalOutput")

    groups_all = list(range(NCORES))
    DIM = c_.DIM

    with tile.TileContext(nc) as tc:
        with tc.tile_pool(name="dram", bufs=1, space="DRAM") as dpool, \
             tc.tile_pool(name="consts", bufs=1) as cpool:
            # DRAM intermediates
            h_loc = [[dpool.tile([c_.SH, DIM], F16, name=f"hloc{l}_{r}")
                      for r in range(3)] for l in range(2)]
            h_full = [[dpool.tile([c_.HROWS, DIM], F16, name=f"hfull{l}_{r}")
                       for r in range(3)] for l in range(2)]
            hcat_dram = [dpool.tile([c_.SH, 3 * DIM], F16, name=f"hcat{l}")
                         for l in range(2)]
            z1_dram = [dpool.tile([c_.SH, DIM], F16, name=f"z1d{l}") for l in range(2)]
            h2_dram = dpool.tile([c_.SH, DIM], F16, name="h2d")
            xcat_dram = dpool.tile([c_.SEGP, 3 * DIM], F32, name="xcatd")
            xcat_red = dpool.tile([c_.SEGP, 3 * DIM], F32, name="xcatr")

            # persistent small consts
            b_sb = cpool.tile([1, 3 * DIM * 2], F16)       # b1|b2 rows
            nc.sync.dma_start(b_sb[:, 0 : 3 * DIM], b1_d[:, :])
            nc.sync.dma_start(b_sb[:, 3 * DIM : 6 * DIM], b2_d[:, :])
            idx_sb = []
            for r in range(3):
                t = cpool.tile([128, TG[r] * 8], I16, name=f"idx{r}")
                nc.sync.dma_start(t[:], gidx_d[r][:, :])
                idx_sb.append(t)

            # ----------------------------------------------------------------
            def allgather(l, r, bsrc):
                nc.gpsimd.collective_compute(
                    "AllGather",
                    ALU.bypass,
                    replica_groups=[groups_all],
                    ins=[h_loc[l][r][:, :]],
                    outs=[h_full[l][r][0 : c_.NPAD, :]],
                )
                # bias row (virtual node) for this layer/relation
                nc.sync.dma_start(
                    h_full[l][r][c_.BIAS_ROW : c_.BIAS_ROW + 1, :],
                    bsrc[:, r * DIM : (r + 1) * DIM],
                )

            def aggregate(l, r, gpool, spool, psp, stp):
                """scatter-add via one-hot matmuls; writes relu result to hcat."""
                gb = 0
                for b in range(c_.NB):
                    nws = [int(Gw[r][c_.WPB * b + k]) for k in range(c_.WPB)]
                    Gb = sum(nws)
                    gt = gpool.tile([128, Gb, DIM], F16, tag="gath")
                    nc.gpsimd.dma_gather(
                        gt[:],
                        h_full[l][r][:, :],
                        idx_sb[r][:, gb * 8 : (gb + Gb) * 8],
                        Gb * 128,
                        Gb * 128,
                        DIM,
                        # single_packet caps at 64 descs/engine = 1024 idxs
                        single_packet=(Gb * 128 <= 1024),
                    )
                    s_sb = spool.tile([128, Gb, c_.WINW], F16, tag="sgt")
                    nc.sync.dma_start(
                        s_sb[:], S_d[r][:, gb * c_.WINW : (gb + Gb) * c_.WINW]
                    )
                    ps = psp.tile([128, DIM], F32, tag="psagg")
                    g = 0
                    for k in range(c_.WPB):
                        w0 = k * c_.WINW
                        for j in range(nws[k]):
                            nc.tensor.matmul(
                                ps[w0 : w0 + c_.WINW, :],
                                s_sb[:, g, :],
                                gt[:, g, :],
                                start=(j == 0),
                                stop=(j == nws[k] - 1),
                            )
                            g += 1
                    st = stp.tile([128, DIM], F16, tag="aggst")
                    nc.vector.tensor_scalar_max(st[:], ps[:], 0.0)  # relu + cast
                    nc.sync.dma_start(
                        hcat_dram[l][b * 128 : (b + 1) * 128, r * DIM : (r + 1) * DIM],
                        st[:],
                    )
                    gb += Gb

            # ================================================================
            # Layer 1: x @ W1_r  (lhsT = xT stripes)
            # ================================================================
            with tc.tile_pool(name="w1p", bufs=1) as w1p, \
                 tc.tile_pool(name="xsp", bufs=2) as xsp, \
                 tc.tile_pool(name="psa", bufs=2, space="PSUM") as psa, \
                 tc.tile_pool(name="hst", bufs=6) as hstp:
                w1_sb = w1p.tile([128, 3 * c_.KC1, DIM], F16)
                for r in range(3):
                    nc.sync.dma_start(
                        w1_sb[:, r * c_.KC1 : (r + 1) * c_.KC1, :],
                        w1_d[r].rearrange("(k p) d -> p k d", p=128),
                    )
                ntps = c_.SUPW // 128
                for ns in range(c_.NSUP):
                    xs_sb = xsp.tile([128, c_.KC1, c_.SUPW], F16, tag="xs")
                    nc.sync.dma_start(
                        xs_sb[:], xT_d[ns].rearrange("(k p) w -> p k w", p=128)
                    )
                    for ntl in range(ntps):
                        nt = ns * ntps + ntl
                        pss = [psa.tile([128, DIM], F32, tag=f"psa{r}", name=f"psa{r}")
                               for r in range(3)]
                        for kc in range(c_.KC1):
                            lhsT = xs_sb[:, kc, ntl * 128 : (ntl + 1) * 128]
                            for r in range(3):
                                nc.tensor.matmul(
                                    pss[r][:],
                                    lhsT,
                                    w1_sb[:, r * c_.KC1 + kc, :],
                                    start=(kc == 0),
                                    stop=(kc == c_.KC1 - 1),
                                )
                        for r in range(3):
                            st = hstp.tile([128, DIM], F16, tag="hstage")
                            nc.vector.tensor_copy(st[:], pss[r][:])
                            nc.sync.dma_start(
                                h_loc[0][r][nt * 128 : (nt + 1) * 128, :], st[:]
                            )

            # ================================================================
            # per-layer: allgather -> aggregate -> MLP
            # ================================================================
            for l in range(2):
                bsrc = b_sb[:, 0 : 3 * DIM] if l == 0 else b_sb[:, 3 * DIM : 6 * DIM]
                for r in range(3):
                    allgather(l, r, bsrc)
                with tc.tile_pool(name=f"gp{l}", bufs=3) as gpool, \
                     tc.tile_pool(name=f"sp{l}", bufs=3) as spool, \
                     tc.tile_pool(name=f"pc{l}", bufs=2, space="PSUM") as psp, \
                     tc.tile_pool(name=f"st{l}", bufs=4) as stp:
                    for r in range(3):
                        aggregate(l, r, gpool, spool, psp, stp)

                # MLP: hcat -> z1 (relu) -> h2
                w1name = m1w1_d if l == 0 else m2w1_d
                w2name = m1w2_d if l == 0 else m2w2_d
                bb1 = m1b1_d if l == 0 else m2b1_d
                bb2 = m1b2_d if l == 0 else m2b2_d
                with tc.tile_pool(name=f"mlp{l}", bufs=1) as mp, \
                     tc.tile_pool(name=f"psm{l}", bufs=2, space="PSUM") as psm, \
                     tc.tile_pool(name=f"mst{l}", bufs=4) as mst:
                    hcatT = mp.tile([128, c_.KCAT, c_.SH], F16)
                    for kt in range(c_.KCAT):
                        nc.sync.dma_start_transpose(
                            hcatT[:, kt, :], hcat_dram[l][:, kt * 128 : (kt + 1) * 128]
                        )
                    mw1_sb = mp.tile([128, c_.KCAT, DIM], F16)
                    nc.sync.dma_start(mw1_sb[:], w1name.ap().rearrange("(k p) d -> p k d", p=128))
                    mw2_sb = mp.tile([128, c_.KD, DIM], F16)
                    nc.sync.dma_start(mw2_sb[:], w2name.ap().rearrange("(k p) d -> p k d", p=128))
                    bb1_sb = mp.tile([128, DIM], F32)
                    nc.sync.dma_start(bb1_sb[:], bb1[:, :])
                    bb2_sb = mp.tile([128, DIM], F32)
                    nc.sync.dma_start(bb2_sb[:], bb2[:, :])

                    for nt in range(c_.NB):
                        ps = psm.tile([128, DIM], F32, tag="psz1")
                        for kc in range(c_.KCAT):
                            nc.tensor.matmul(
                                ps[:],
                                hcatT[:, kc, nt * 128 : (nt + 1) * 128],
                                mw1_sb[:, kc, :],
                                start=(kc == 0),
                                stop=(kc == c_.KCAT - 1),
                            )
                        tmp = mst.tile([128, DIM], F32, tag="ztmp")
                        nc.vector.tensor_add(tmp[:], ps[:], bb1_sb[:])
                        z1st = mst.tile([128, DIM], F16, tag="z1st")
                        nc.vector.tensor_scalar_max(z1st[:], tmp[:], 0.0)
                        nc.sync.dma_start(
                            z1_dram[l][nt * 128 : (nt + 1) * 128, :], z1st[:]
                        )
                    z1T = mp.tile([128, c_.KD, c_.SH], F16)
                    for kt in range(c_.KD):
                        nc.sync.dma_start_transpose(
                            z1T[:, kt, :], z1_dram[l][:, kt * 128 : (kt + 1) * 128]
                        )
                    if l == 0:
                        for nt in range(c_.NB):
                            ps = psm.tile([128, DIM], F32, tag="psz2")
                            for kc in range(c_.KD):
                                nc.tensor.matmul(
                                    ps[:],
                                    z1T[:, kc, nt * 128 : (nt + 1) * 128],
                                    mw2_sb[:, kc, :],
                                    start=(kc == 0),
                                    stop=(kc == c_.KD - 1),
                                )
                            h2st = mst.tile([128, DIM], F16, tag="h2st")
                            nc.vector.tensor_add(h2st[:], ps[:], bb2_sb[:])
                            nc.sync.dma_start(
                                h2_dram[nt * 128 : (nt + 1) * 128, :], h2st[:]
                            )
                        # layer-2 GCN matmuls: h2 @ W2_r
                        h2T = mp.tile([128, c_.KD, c_.SH], F16)
                        for kt in range(c_.KD):
                            nc.sync.dma_start_transpose(
                                h2T[:, kt, :], h2_dram[:, kt * 128 : (kt + 1) * 128]
                            )
                        w2_sb = mp.tile([128, 3 * c_.KD, DIM], F16)
                        for r in range(3):
                            nc.sync.dma_start(
                                w2_sb[:, r * c_.KD : (r + 1) * c_.KD, :],
                                w2_d[r].rearrange("(k p) d -> p k d", p=128),
                            )
                        for nt in range(c_.NB):
                            pss = [psm.tile([128, DIM], F32, tag=f"psg{r}",
                                            name=f"psg{r}", bufs=1) for r in range(3)]
                            for kc in range(c_.KD):
                                lhsT = h2T[:, kc, nt * 128 : (nt + 1) * 128]
                                for r in range(3):
                                    nc.tensor.matmul(
                                        pss[r][:],
                                        lhsT,
                                        w2_sb[:, r * c_.KD + kc, :],
                                        start=(kc == 0),
                                        stop=(kc == c_.KD - 1),
                                    )
                            for r in range(3):
                                st = mst.tile([128, DIM], F16, tag="hstage2")
                                nc.vector.tensor_copy(st[:], pss[r][:])
                                nc.sync.dma_start(
                                    h_loc[1][r][nt * 128 : (nt + 1) * 128, :], st[:]
                                )
                    else:
                        # final node features h3 (kept in SBUF)
                        h3_sb = cpool.tile([128, c_.NB, DIM], F16)
                        for nt in range(c_.NB):
                            ps = psm.tile([128, DIM], F32, tag="psz2")
                            for kc in range(c_.KD):
                                nc.tensor.matmul(
                                    ps[:],
                                    z1T[:, kc, nt * 128 : (nt + 1) * 128],
                                    mw2_sb[:, kc, :],
                                    start=(kc == 0),
                                    stop=(kc == c_.KD - 1),
                                )
                            nc.vector.tensor_add(h3_sb[:, nt, :], ps[:], bb2_sb[:])

            # ================================================================
            # segment-mean pooling + AllReduce
            # ================================================================
            with tc.tile_pool(name="segp", bufs=1) as sgp, \
                 tc.tile_pool(name="psx", bufs=1, space="PSUM") as psx:
                P1_sb = sgp.tile([128, c_.NB, c_.SEGP], F16)
                nc.sync.dma_start(P1_sb[:], P1_d[:, :, :])
                P3_sb = sgp.tile([128, c_.NB, c_.SEGP], F16)
                nc.sync.dma_start(P3_sb[:], P3_d[:, :, :])
                x1a = psx.tile([128, DIM], F32, tag="x1a", bufs=1)
                x1b = psx.tile([64, DIM], F32, tag="x1b", bufs=1)
                x3a = psx.tile([128, DIM], F32, tag="x3a", bufs=1)
                x3b = psx.tile([64, DIM], F32, tag="x3b", bufs=1)
                for nt in range(c_.NB):
                    rhs = h3_sb[:, nt, :]
                    s0, s1 = (nt == 0), (nt == c_.NB - 1)
                    nc.tensor.matmul(x1a[:], P1_sb[:, nt, 0:128], rhs, start=s0, stop=s1)
                    nc.tensor.matmul(x1b[:], P1_sb[:, nt, 128:192], rhs, start=s0, stop=s1)
                    nc.tensor.matmul(x3a[:], P3_sb[:, nt, 0:128], rhs, start=s0, stop=s1)
                    nc.tensor.matmul(x3b[:], P3_sb[:, nt, 128:192], rhs, start=s0, stop=s1)
                xc1 = sgp.tile([128, 3 * DIM], F32)
                xc2 = sgp.tile([64, 3 * DIM], F32)
                nc.vector.tensor_copy(xc1[:, 0:DIM], x1a[:])
                nc.vector.tensor_copy(xc1[:, DIM : 2 * DIM], x3a[:])
                nc.vector.tensor_copy(xc1[:, 2 * DIM : 3 * DIM], x3a[:])
                nc.vector.tensor_copy(xc2[:, 0:DIM], x1b[:])
                nc.vector.tensor_copy(xc2[:, DIM : 2 * DIM], x3b[:])
                nc.vector.tensor_copy(xc2[:, 2 * DIM : 3 * DIM], x3b[:])
                nc.sync.dma_start(xcat_dram[0:128, :], xc1[:])
                nc.sync.dma_start(xcat_dram[128 : c_.SEGP, :], xc2[:])
            nc.gpsimd.collective_compute(
                "AllReduce",
                ALU.add,
                replica_groups=[groups_all],
                ins=[xcat_dram[:, :]],
                outs=[xcat_red[:, :]],
            )

            # ================================================================
            # final MLP (fp32) + log_softmax, replicated
            # ================================================================
            with tc.tile_pool(name="fin", bufs=1) as fp, \
                 tc.tile_pool(name="psf", bufs=2, space="PSUM") as psf:
                ident_sb = fp.tile([128, 128], F32)
                nc.sync.dma_start(ident_sb[:], ident_d[:, :])
                xr1 = fp.tile([128, 3 * DIM], F32)
                nc.sync.dma_start(xr1[:], xcat_red[0:128, :])
                xr2 = fp.tile([64, 3 * DIM], F32)
                nc.sync.dma_start(xr2[:], xcat_red[128 : c_.SEGP, :])
                xcT = fp.tile([128, c_.KCAT, c_.SEGP], F32)
                for kc in range(c_.KCAT):
                    pt = psf.tile([128, 128], F32, tag="ptr")
                    nc.tensor.transpose(pt[:], xr1[:, kc * 128 : (kc + 1) * 128], ident_sb[:])
                    nc.vector.tensor_copy(xcT[:, kc, 0:128], pt[:])
                    pt2 = psf.tile([128, 64], F32, tag="ptr2", bufs=1)
                    nc.tensor.transpose(
                        pt2[:], xr2[:, kc * 128 : (kc + 1) * 128], ident_sb[0:64, 0:64]
                    )
                    nc.vector.tensor_copy(xcT[:, kc, 128 : c_.SEGP], pt2[:])
                mw1_sb = fp.tile([128, c_.KCAT, DIM], F32)
                nc.sync.dma_start(mw1_sb[:], mw1_d.ap().rearrange("(k p) d -> p k d", p=128))
                mb1_sb = fp.tile([128, DIM], F32)
                nc.sync.dma_start(mb1_sb[:], mb1_d[:, :])
                y1a = psf.tile([128, DIM], F32, tag="y1a", bufs=1)
                y1b = psf.tile([64, DIM], F32, tag="y1b", bufs=1)
                for kc in range(c_.KCAT):
                    s0, s1 = (kc == 0), (kc == c_.KCAT - 1)
                    nc.tensor.matmul(y1a[:], xcT[:, kc, 0:128], mw1_sb[:, kc, :], start=s0, stop=s1)
                    nc.tensor.matmul(y1b[:], xcT[:, kc, 128:192], mw1_sb[:, kc, :], start=s0, stop=s1)
                y1s1 = fp.tile([128, DIM], F32)
                nc.vector.tensor_add(y1s1[:], y1a[:], mb1_sb[:])
                nc.vector.tensor_scalar_max(y1s1[:], y1s1[:], 0.0)
                y1s2 = fp.tile([64, DIM], F32)
                nc.vector.tensor_add(y1s2[:], y1b[:], mb1_sb[0:64, :])
                nc.vector.tensor_scalar_max(y1s2[:], y1s2[:], 0.0)
                y1T = fp.tile([128, c_.KD, c_.SEGP], F32)
                for kc in range(c_.KD):
                    pt = psf.tile([128, 128], F32, tag="ptr")
                    nc.tensor.transpose(pt[:], y1s1[:, kc * 128 : (kc + 1) * 128], ident_sb[:])
                    nc.vector.tensor_copy(y1T[:, kc, 0:128], pt[:])
                    pt2 = psf.tile([128, 64], F32, tag="ptr2", bufs=1)
                    nc.tensor.transpose(
                        pt2[:], y1s2[:, kc * 128 : (kc + 1) * 128], ident_sb[0:64, 0:64]
                    )
                    nc.vector.tensor_copy(y1T[:, kc, 128 : c_.SEGP], pt2[:])
                mw2_sb = fp.tile([128, c_.KD, c_.C], F32)
                nc.sync.dma_start(mw2_sb[:], mw2_d.ap().rearrange("(k p) d -> p k d", p=128))
                mb2_sb = fp.tile([128, c_.C], F32)
                nc.sync.dma_start(mb2_sb[:], mb2_d[:, :])
                la = psf.tile([128, c_.C], F32, tag="la", bufs=1)
                lb = psf.tile([64, c_.C], F32, tag="lb", bufs=1)
                for kc in range(c_.KD):
                    s0, s1 = (kc == 0), (kc == c_.KD - 1)
                    nc.tensor.matmul(la[:], y1T[:, kc, 0:128], mw2_sb[:, kc, :], start=s0, stop=s1)
                    nc.tensor.matmul(lb[:], y1T[:, kc, 128:192], mw2_sb[:, kc, :], start=s0, stop=s1)
                outs = []
                for ps_, bias_, np_ in ((la[:], mb2_sb[:], 128), (lb[:], mb2_sb[0:64, :], 64)):
                    lg = fp.tile([np_, c_.C], F32, tag=f"lg{np_}", name=f"lg{np_}")
                    nc.vector.tensor_add(lg[:], ps_, bias_)
                    mx = fp.tile([np_, 1], F32, tag=f"mx{np_}")
                    nc.vector.tensor_reduce(mx[:], lg[:], AX.X, ALU.max)
                    tt = fp.tile([np_, c_.C], F32, tag=f"tt{np_}")
                    nc.vector.tensor_scalar(tt[:], lg[:], mx[:], None, ALU.subtract)
                    ex = fp.tile([np_, c_.C], F32, tag=f"ex{np_}")
                    nc.scalar.activation(ex[:], tt[:], AF.Exp)
                    sm = fp.tile([np_, 1], F32, tag=f"sm{np_}")
                    nc.vector.tensor_reduce(sm[:], ex[:], AX.X, ALU.add)
                    ln = fp.tile([np_, 1], F32, tag=f"ln{np_}")
                    nc.scalar.activation(ln[:], sm[:], AF.Ln)
                    lp = fp.tile([np_, c_.C], F32, tag=f"lp{np_}")
                    nc.vector.tensor_scalar(lp[:], tt[:], ln[:], None, ALU.subtract)
                    outs.append(lp)
                nc.sync.dma_start(out_d[0:128, :], outs[0][:])
                nc.sync.dma_start(out_d[128 : c_.NORIG, :], outs[1][0 : c_.NORIG - 128, :])

    nc.compile()
    return nc


# --------------------------------------------------------------------------
# entry point
# --------------------------------------------------------------------------

def kernel(**inputs):
    meta, in_maps = prep(inputs, DEFAULT_CFG)
    nc = build(meta, DEFAULT_CFG)
    res = bass_utils.run_bass_kernel_spmd(nc, in_maps, core_ids=list(range(NCORES)))
    out = np.asarray(res.results[0]["out"], np.float32)
    return out


# revision 21
# speedup vs baseline: 1.0677x; 1.0677x over previous
"""Trainium2 Bass kernel for a 3-relation GCN (gnn_message_passing).

Strategy (8 NeuronCores, SPMD):
  - Node dim (20000 -> padded 20480) sharded 2560/core.
  - Layer GCN matmuls computed locally; per-relation [2560,256] fp16 shards
    AllGathered to a full node-feature table in DRAM.
  - Edge aggregation: edges partitioned by destination (host side), sorted,
    chunked into 128-edge groups per 64-dst window.  h rows are fetched with
    gpsimd dma_gather; a host-built one-hot scatter matrix S (carrying the
    symmetric GCN norms + bias edges) turns the scatter-add into PE matmuls
    accumulating in PSUM.
  - MLPs run from DMA-transposed (xbar) activations so the contraction dim
    lands on partitions.
  - Segment-mean pooling is a one-hot (1/cnt) matmul; the [192,768] fp32
    partials are AllReduced; the tiny final MLP + log_softmax is replicated.
All device math fp16 with fp32 PSUM accumulation; final stage fp32.
"""

import sys

import numpy as np

sys.path.insert(0, "/opt/trn_rl_repo")

import concourse.bacc as bacc  # noqa: E402
import concourse.bass as bass  # noqa: E402
import concourse.mybir as mybir  # noqa: E402
import concourse.tile as tile  # noqa: E402
from concourse import bass_utils  # noqa: E402

F16 = mybir.dt.float16
F32 = mybir.dt.float32
I16 = mybir.dt.int16
AF = mybir.ActivationFunctionType
ALU = mybir.AluOpType
AX = mybir.AxisListType

NCORES = 8


class Cfg:
    def __init__(self, n=20000, e=320000, f_in=5189, dim=256, norig=183, ncls=7,
                 npad=20480, fpad=5248, winw=64, nsup=10):
        self.N = n
        self.E = e
        self.F_IN = f_in
        self.DIM = dim
        self.NORIG = norig
        self.C = ncls
        self.NPAD = npad
        self.SH = npad // NCORES          # nodes per core
        self.NB = self.SH // 128          # 128-dst blocks per core
        self.WINW = winw                  # dst window width for scatter matmuls
        self.NWIN = self.SH // winw
        self.WPB = 128 // winw            # windows per block
        self.FPAD = fpad
        self.KC1 = fpad // 128
        self.NSUP = nsup                  # x stripes
        self.SUPW = self.SH // nsup
        self.BIAS_ROW = npad
        self.HROWS = npad + 1
        self.SEGP = 192                   # padded NORIG
        self.KCAT = 3 * dim // 128        # 6
        self.KD = dim // 128              # 2
        assert self.SH % 128 == 0 and self.SUPW % 128 == 0 and fpad % 128 == 0


DEFAULT_CFG = Cfg()


# --------------------------------------------------------------------------
# host-side preprocessing
# --------------------------------------------------------------------------

def _wrap_idx(idx, cfg):
    """dma_gather index layout: pos i -> partition i%16, col i//16; the 16-row
    block replicated across the 8 GPSIMD cores (128 partitions)."""
    tg8 = idx.shape[0] // 16
    w = idx.astype(np.int16).reshape(tg8, 16).T            # [16, tg*8]
    return np.ascontiguousarray(np.tile(w, (8, 1)))        # [128, tg*8]


def prep(inputs, cfg=DEFAULT_CFG):
    c_ = cfg
    f16 = np.float16
    x = np.asarray(inputs["x"], np.float32)
    w1 = np.asarray(inputs["w1"], np.float32)
    b1 = np.asarray(inputs["b1"], np.float32)
    w2 = np.asarray(inputs["w2"], np.float32)
    b2 = np.asarray(inputs["b2"], np.float32)
    eis = [np.asarray(inputs[f"edge_index_{k}"], np.int64) for k in (1, 2, 3)]
    idx1 = np.asarray(inputs["index_1"], np.int64)
    idx3 = np.asarray(inputs["index_3"], np.int64)

    # ---- per-relation edge prep: self loops, degrees, norms, dst partition
    Gw = np.zeros((3, c_.NWIN), np.int64)   # groups per (relative) window, max over cores
    core_edges = [[None] * 3 for _ in range(NCORES)]
    loop = np.arange(c_.N)
    for r in range(3):
        src = np.concatenate([eis[r][0], loop])
        dst = np.concatenate([eis[r][1], loop])
        deg = np.bincount(dst, minlength=c_.N).astype(np.float64)
        dinv = 1.0 / np.sqrt(deg)
        norm = (dinv[src] * dinv[dst]).astype(np.float32)
        for c in range(NCORES):
            lo = c * c_.SH
            m = (dst >= lo) & (dst < lo + c_.SH)
            es, ed, en = src[m], dst[m] - lo, norm[m]
            nreal = int(min(max(c_.N - lo, 0), c_.SH))     # real dst rows in shard
            es = np.concatenate([es, np.full(nreal, c_.BIAS_ROW, np.int64)])
            ed = np.concatenate([ed, np.arange(nreal)])
            en = np.concatenate([en, np.ones(nreal, np.float32)])
            o = np.argsort(ed, kind="stable")
            es, ed, en = es[o], ed[o], en[o]
            core_edges[c][r] = (es, ed, en)
            cnts = np.bincount(ed // c_.WINW, minlength=c_.NWIN)
            Gw[r] = np.maximum(Gw[r], (cnts + 127) // 128)
    Gw = np.maximum(Gw, 1)  # every window needs >=1 group so its PSUM is written
    TG = Gw.sum(axis=1)

    # ---- pooling one-hot (1/cnt) matrices, global counts
    cnt1 = np.bincount(idx1, minlength=c_.NORIG).astype(np.float64)
    cnt3 = np.bincount(idx3, minlength=c_.NORIG).astype(np.float64)
    r1 = (1.0 / np.maximum(cnt1, 1.0)).astype(np.float32)
    r3 = (1.0 / np.maximum(cnt3, 1.0)).astype(np.float32)
    P1 = np.zeros((NCORES, 128, c_.NB, c_.SEGP), f16)
    P3 = np.zeros((NCORES, 128, c_.NB, c_.SEGP), f16)
    nn = np.arange(c_.N)
    cc, ll = nn // c_.SH, nn % c_.SH
    P1[cc, ll % 128, ll // 128, idx1] = r1[idx1].astype(f16)
    P3[cc, ll % 128, ll // 128, idx3] = r3[idx3].astype(f16)

    # ---- shared (replicated) tensors
    w1h = np.zeros((3, c_.FPAD, c_.DIM), f16)
    w1h[:, : c_.F_IN] = w1.astype(f16)
    shared = {
        "w1h": w1h,
        "w2h": w2.astype(f16),
        "b1row": b1.astype(f16).reshape(1, -1),
        "b2row": b2.astype(f16).reshape(1, -1),
        "m1w1h": np.asarray(inputs["m1w1"], np.float32).astype(f16),
        "m1w2h": np.asarray(inputs["m1w2"], np.float32).astype(f16),
        "m2w1h": np.asarray(inputs["m2w1"], np.float32).astype(f16),
        "m2w2h": np.asarray(inputs["m2w2"], np.float32).astype(f16),
        "m1b1bc": np.tile(np.asarray(inputs["m1b1"], np.float32), (128, 1)),
        "m1b2bc": np.tile(np.asarray(inputs["m1b2"], np.float32), (128, 1)),
        "m2b1bc": np.tile(np.asarray(inputs["m2b1"], np.float32), (128, 1)),
        "m2b2bc": np.tile(np.asarray(inputs["m2b2"], np.float32), (128, 1)),
        "mw1f": np.asarray(inputs["mw1"], np.float32),
        "mw2f": np.asarray(inputs["mw2"], np.float32),
        "mb1bc": np.tile(np.asarray(inputs["mb1"], np.float32), (128, 1)),
        "mb2bc": np.tile(np.asarray(inputs["mb2"], np.float32), (128, 1)),
        "ident": np.eye(128, dtype=np.float32),
    }

    # ---- per-core tensors
    in_maps = []
    for c in range(NCORES):
        d = dict(shared)
        lo = c * c_.SH
        nreal = int(min(max(c_.N - lo, 0), c_.SH))
        xs = np.zeros((c_.SH, c_.FPAD), f16)
        xs[:nreal, : c_.F_IN] = x[lo : lo + nreal].astype(f16)
        xT = np.ascontiguousarray(
            xs.T.reshape(c_.FPAD, c_.NSUP, c_.SUPW).transpose(1, 0, 2)
        )
        d["xT"] = xT
        for r in range(3):
            es, ed, en = core_edges[c][r]
            win = ed // c_.WINW
            tg = int(TG[r])
            sp = np.full(tg * 128, c_.BIAS_ROW, np.int64)
            dl = np.zeros(tg * 128, np.int64)
            nm = np.zeros(tg * 128, np.float32)
            st = np.searchsorted(win, np.arange(c_.NWIN), side="left")
            en_ = np.searchsorted(win, np.arange(c_.NWIN), side="right")
            g0 = 0
            for w in range(c_.NWIN):
                cnt = int(en_[w] - st[w])
                sl = slice(g0 * 128, g0 * 128 + cnt)
                sp[sl] = es[st[w] : en_[w]]
                dl[sl] = ed[st[w] : en_[w]] - w * c_.WINW
                nm[sl] = en[st[w] : en_[w]]
                g0 += int(Gw[r][w])
            S = np.zeros((tg * 128, c_.WINW), f16)
            S[np.arange(tg * 128), dl] = nm.astype(f16)  # pad rows write 0.0
            d[f"S_{r}"] = np.ascontiguousarray(
                S.reshape(tg, 128, c_.WINW).transpose(1, 0, 2).reshape(128, tg * c_.WINW)
            )
            d[f"gidx_{r}"] = _wrap_idx(sp, c_)
        d["P1"] = np.ascontiguousarray(P1[c])
        d["P3"] = np.ascontiguousarray(P3[c])
        in_maps.append(d)

    meta = {"Gw": Gw.astype(int), "TG": [int(t) for t in TG]}
    return meta, in_maps


# --------------------------------------------------------------------------
# device program
# --------------------------------------------------------------------------

def build(meta, cfg=DEFAULT_CFG, enable_asserts=False):
    c_ = cfg
    Gw, TG = meta["Gw"], meta["TG"]
    nc = bacc.Bacc(
        "TRN2",
        target_bir_lowering=False,
        debug=False,
        enable_asserts=enable_asserts,
        num_devices=NCORES,
        num_swdge_queues=4,
    )

    # ---------------- DRAM I/O
    xT_d = nc.dram_tensor("xT", [c_.NSUP, c_.FPAD, c_.SUPW], F16, kind="ExternalInput")
    w1_d = nc.dram_tensor("w1h", [3, c_.FPAD, c_.DIM], F16, kind="ExternalInput")
    w2_d = nc.dram_tensor("w2h", [3, c_.DIM, c_.DIM], F16, kind="ExternalInput")
    b1_d = nc.dram_tensor("b1row", [1, 3 * c_.DIM], F16, kind="ExternalInput")
    b2_d = nc.dram_tensor("b2row", [1, 3 * c_.DIM], F16, kind="ExternalInput")
    m1w1_d = nc.dram_tensor("m1w1h", [3 * c_.DIM, c_.DIM], F16, kind="ExternalInput")
    m1w2_d = nc.dram_tensor("m1w2h", [c_.DIM, c_.DIM], F16, kind="ExternalInput")
    m2w1_d = nc.dram_tensor("m2w1h", [3 * c_.DIM, c_.DIM], F16, kind="ExternalInput")
    m2w2_d = nc.dram_tensor("m2w2h", [c_.DIM, c_.DIM], F16, kind="ExternalInput")
    m1b1_d = nc.dram_tensor("m1b1bc", [128, c_.DIM], F32, kind="ExternalInput")
    m1b2_d = nc.dram_tensor("m1b2bc", [128, c_.DIM], F32, kind="ExternalInput")
    m2b1_d = nc.dram_tensor("m2b1bc", [128, c_.DIM], F32, kind="ExternalInput")
    m2b2_d = nc.dram_tensor("m2b2bc", [128, c_.DIM], F32, kind="ExternalInput")
    mw1_d = nc.dram_tensor("mw1f", [3 * c_.DIM, c_.DIM], F32, kind="ExternalInput")
    mw2_d = nc.dram_tensor("mw2f", [c_.DIM, c_.C], F32, kind="ExternalInput")
    mb1_d = nc.dram_tensor("mb1bc", [128, c_.DIM], F32, kind="ExternalInput")
    mb2_d = nc.dram_tensor("mb2bc", [128, c_.C], F32, kind="ExternalInput")
    ident_d = nc.dram_tensor("ident", [128, 128], F32, kind="ExternalInput")
    S_d = [
        nc.dram_tensor(f"S_{r}", [128, TG[r] * c_.WINW], F16, kind="ExternalInput")
        for r in range(3)
    ]
    gidx_d = [
        nc.dram_tensor(f"gidx_{r}", [128, TG[r] * 8], I16, kind="ExternalInput")
        for r in range(3)
    ]
    P1_d = nc.dram_tensor("P1", [128, c_.NB, c_.SEGP], F16, kind="ExternalInput")
    P3_d = nc.dram_tensor("P3", [128, c_.NB, c_.SEGP], F16, kind="ExternalInput")
    out_d = nc.dram_tensor("out", [c_.NORIG, c_.C], F32, kind="ExternalOutput")

    groups_all = list(range(NCORES))
    DIM = c_.DIM

    with tile.TileContext(nc) as tc:
        with tc.tile_pool(name="dram", bufs=1, space="DRAM") as dpool, \
             tc.tile_pool(name="consts", bufs=1) as cpool:
            # DRAM intermediates
            h_loc = [[dpool.tile([c_.SH, DIM], F16, name=f"hloc{l}_{r}")
                      for r in range(3)] for l in range(2)]
            h_full = [[dpool.tile([c_.HROWS, DIM], F16, name=f"hfull{l}_{r}")
                       for r in range(3)] for l in range(2)]
            hcat_dram = [dpool.tile([c_.SH, 3 * DIM], F16, name=f"hcat{l}")
                         for l in range(2)]
            z1_dram = [dpool.tile([c_.SH, DIM], F16, name=f"z1d{l}") for l in range(2)]
            h2_dram = dpool.tile([c_.SH, DIM], F16, name="h2d")
            xcat_dram = dpool.tile([c_.SEGP, 3 * DIM], F32, name="xcatd")
            xcat_red = dpool.tile([c_.SEGP, 3 * DIM], F32, name="xcatr")

            # persistent small consts
            b_sb = cpool.tile([1, 3 * DIM * 2], F16)       # b1|b2 rows
            nc.sync.dma_start(b_sb[:, 0 : 3 * DIM], b1_d[:, :])
            nc.sync.dma_start(b_sb[:, 3 * DIM : 6 * DIM], b2_d[:, :])
            idx_sb = []
            for r in range(3):
                t = cpool.tile([128, TG[r] * 8], I16, name=f"idx{r}")
                nc.sync.dma_start(t[:], gidx_d[r][:, :])
                idx_sb.append(t)

            # ----------------------------------------------------------------
            def allgather(l, r, bsrc):
                nc.gpsimd.collective_compute(
                    "AllGather",
                    ALU.bypass,
                    replica_groups=[groups_all],
                    ins=[h_loc[l][r][:, :]],
                    outs=[h_full[l][r][0 : c_.NPAD, :]],
                )
                # bias row (virtual node) for this layer/relation
                nc.sync.dma_start(
                    h_full[l][r][c_.BIAS_ROW : c_.BIAS_ROW + 1, :],
                    bsrc[:, r * DIM : (r + 1) * DIM],
                )

            def aggregate(l, r, gpool, spool, psp, stp):
                """scatter-add via one-hot matmuls; writes relu result to hcat."""
                gb = 0
                for b in range(c_.NB):
                    qn = b % 4
                    nws = [int(Gw[r][c_.WPB * b + k]) for k in range(c_.WPB)]
                    Gb = sum(nws)
                    gt = gpool.tile([128, Gb, DIM], F16, tag="gath")
                    nc.gpsimd.dma_gather(
                        gt[:],
                        h_full[l][r][:, :],
                        idx_sb[r][:, gb * 8 : (gb + Gb) * 8],
                        Gb * 128,
                        Gb * 128,
                        DIM,
                        # single_packet caps at 64 descs/engine = 1024 idxs
                        single_packet=(Gb * 128 <= 1024),
                        queue_num=qn,
                    )
                    s_sb = spool.tile([128, Gb, c_.WINW], F16, tag="sgt")
                    nc.sync.dma_start(
                        s_sb[:], S_d[r][:, gb * c_.WINW : (gb + Gb) * c_.WINW]
                    )
                    ps = psp.tile([128, DIM], F32, tag="psagg")
                    g = 0
                    for k in range(c_.WPB):
                        w0 = k * c_.WINW
                        for j in range(nws[k]):
                            nc.tensor.matmul(
                                ps[w0 : w0 + c_.WINW, :],
                                s_sb[:, g, :],
                                gt[:, g, :],
                                start=(j == 0),
                                stop=(j == nws[k] - 1),
                            )
                            g += 1
                    st = stp.tile([128, DIM], F16, tag="aggst")
                    nc.vector.tensor_scalar_max(st[:], ps[:], 0.0)  # relu + cast
                    nc.sync.dma_start(
                        hcat_dram[l][b * 128 : (b + 1) * 128, r * DIM : (r + 1) * DIM],
                        st[:],
                    )
                    gb += Gb

            # ================================================================
            # Layer 1: x @ W1_r  (lhsT = xT stripes)
            # ================================================================
            with tc.tile_pool(name="w1p", bufs=1) as w1p, \
                 tc.tile_pool(name="xsp", bufs=2) as xsp, \
                 tc.tile_pool(name="psa", bufs=2, space="PSUM") as psa, \
                 tc.tile_pool(name="hst", bufs=6) as hstp:
                w1_sb = w1p.tile([128, 3 * c_.KC1, DIM], F16)
                for r in range(3):
                    nc.sync.dma_start(
                        w1_sb[:, r * c_.KC1 : (r + 1) * c_.KC1, :],
                        w1_d[r].rearrange("(k p) d -> p k d", p=128),
                    )
                ntps = c_.SUPW // 128
                for ns in range(c_.NSUP):
                    xs_sb = xsp.tile([128, c_.KC1, c_.SUPW], F16, tag="xs")
                    nc.sync.dma_start(
                        xs_sb[:], xT_d[ns].rearrange("(k p) w -> p k w", p=128)
                    )
                    for ntl in range(ntps):
                        nt = ns * ntps + ntl
                        pss = [psa.tile([128, DIM], F32, tag=f"psa{r}", name=f"psa{r}")
                               for r in range(3)]
                        for kc in range(c_.KC1):
                            lhsT = xs_sb[:, kc, ntl * 128 : (ntl + 1) * 128]
                            for r in range(3):
                                nc.tensor.matmul(
                                    pss[r][:],
                                    lhsT,
                                    w1_sb[:, r * c_.KC1 + kc, :],
                                    start=(kc == 0),
                                    stop=(kc == c_.KC1 - 1),
                                )
                        for r in range(3):
                            st = hstp.tile([128, DIM], F16, tag="hstage")
                            nc.vector.tensor_copy(st[:], pss[r][:])
                            nc.sync.dma_start(
                                h_loc[0][r][nt * 128 : (nt + 1) * 128, :], st[:]
                            )

            # ================================================================
            # per-layer: allgather -> aggregate -> MLP
            # ================================================================
            for l in range(2):
                bsrc = b_sb[:, 0 : 3 * DIM] if l == 0 else b_sb[:, 3 * DIM : 6 * DIM]
                for r in range(3):
                    allgather(l, r, bsrc)
                with tc.tile_pool(name=f"gp{l}", bufs=3) as gpool, \
                     tc.tile_pool(name=f"sp{l}", bufs=3) as spool, \
                     tc.tile_pool(name=f"pc{l}", bufs=2, space="PSUM") as psp, \
                     tc.tile_pool(name=f"st{l}", bufs=4) as stp:
                    for r in range(3):
                        aggregate(l, r, gpool, spool, psp, stp)

                # MLP: hcat -> z1 (relu) -> h2
                w1name = m1w1_d if l == 0 else m2w1_d
                w2name = m1w2_d if l == 0 else m2w2_d
                bb1 = m1b1_d if l == 0 else m2b1_d
                bb2 = m1b2_d if l == 0 else m2b2_d
                with tc.tile_pool(name=f"mlp{l}", bufs=1) as mp, \
                     tc.tile_pool(name=f"psm{l}", bufs=2, space="PSUM") as psm, \
                     tc.tile_pool(name=f"mst{l}", bufs=4) as mst:
                    hcatT = mp.tile([128, c_.KCAT, c_.SH], F16)
                    for kt in range(c_.KCAT):
                        nc.sync.dma_start_transpose(
                            hcatT[:, kt, :], hcat_dram[l][:, kt * 128 : (kt + 1) * 128]
                        )
                    mw1_sb = mp.tile([128, c_.KCAT, DIM], F16)
                    nc.sync.dma_start(mw1_sb[:], w1name.ap().rearrange("(k p) d -> p k d", p=128))
                    mw2_sb = mp.tile([128, c_.KD, DIM], F16)
                    nc.sync.dma_start(mw2_sb[:], w2name.ap().rearrange("(k p) d -> p k d", p=128))
                    bb1_sb = mp.tile([128, DIM], F32)
                    nc.sync.dma_start(bb1_sb[:], bb1[:, :])
                    bb2_sb = mp.tile([128, DIM], F32)
                    nc.sync.dma_start(bb2_sb[:], bb2[:, :])

                    for nt in range(c_.NB):
                        ps = psm.tile([128, DIM], F32, tag="psz1")
                        for kc in range(c_.KCAT):
                            nc.tensor.matmul(
                                ps[:],
                                hcatT[:, kc, nt * 128 : (nt + 1) * 128],
                                mw1_sb[:, kc, :],
                                start=(kc == 0),
                                stop=(kc == c_.KCAT - 1),
                            )
                        tmp = mst.tile([128, DIM], F32, tag="ztmp")
                        nc.vector.tensor_add(tmp[:], ps[:], bb1_sb[:])
                        z1st = mst.tile([128, DIM], F16, tag="z1st")
                        nc.vector.tensor_scalar_max(z1st[:], tmp[:], 0.0)
                        nc.sync.dma_start(
                            z1_dram[l][nt * 128 : (nt + 1) * 128, :], z1st[:]
                        )
                    z1T = mp.tile([128, c_.KD, c_.SH], F16)
                    for kt in range(c_.KD):
                        nc.sync.dma_start_transpose(
                            z1T[:, kt, :], z1_dram[l][:, kt * 128 : (kt + 1) * 128]
                        )
                    if l == 0:
                        for nt in range(c_.NB):
                            ps = psm.tile([128, DIM], F32, tag="psz2")
                            for kc in range(c_.KD):
                                nc.tensor.matmul(
                                    ps[:],
                                    z1T[:, kc, nt * 128 : (nt + 1) * 128],
                                    mw2_sb[:, kc, :],
                                    start=(kc == 0),
                                    stop=(kc == c_.KD - 1),
                                )
                            h2st = mst.tile([128, DIM], F16, tag="h2st")
                            nc.vector.tensor_add(h2st[:], ps[:], bb2_sb[:])
                            nc.sync.dma_start(
                                h2_dram[nt * 128 : (nt + 1) * 128, :], h2st[:]
                            )
                        # layer-2 GCN matmuls: h2 @ W2_r
                        h2T = mp.tile([128, c_.KD, c_.SH], F16)
                        for kt in range(c_.KD):
                            nc.sync.dma_start_transpose(
                                h2T[:, kt, :], h2_dram[:, kt * 128 : (kt + 1) * 128]
                            )
                        w2_sb = mp.tile([128, 3 * c_.KD, DIM], F16)
                        for r in range(3):
                            nc.sync.dma_start(
                                w2_sb[:, r * c_.KD : (r + 1) * c_.KD, :],
                                w2_d[r].rearrange("(k p) d -> p k d", p=128),
                            )
                        for nt in range(c_.NB):
                            pss = [psm.tile([128, DIM], F32, tag=f"psg{r}",
                                            name=f"psg{r}", bufs=1) for r in range(3)]
                            for kc in range(c_.KD):
                                lhsT = h2T[:, kc, nt * 128 : (nt + 1) * 128]
                                for r in range(3):
                                    nc.tensor.matmul(
                                        pss[r][:],
                                        lhsT,
                                        w2_sb[:, r * c_.KD + kc, :],
                                        start=(kc == 0),
                                        stop=(kc == c_.KD - 1),
                                    )
                            for r in range(3):
                                st = mst.tile([128, DIM], F16, tag="hstage2")
                                nc.vector.tensor_copy(st[:], pss[r][:])
                                nc.sync.dma_start(
                                    h_loc[1][r][nt * 128 : (nt + 1) * 128, :], st[:]
                                )
                    else:
                        # final node features h3 (kept in SBUF)
                        h3_sb = cpool.tile([128, c_.NB, DIM], F16)
                        for nt in range(c_.NB):
                            ps = psm.tile([128, DIM], F32, tag="psz2")
                            for kc in range(c_.KD):
                                nc.tensor.matmul(
                                    ps[:],
                                    z1T[:, kc, nt * 128 : (nt + 1) * 128],
                                    mw2_sb[:, kc, :],
                                    start=(kc == 0),
                                    stop=(kc == c_.KD - 1),
                                )
                            nc.vector.tensor_add(h3_sb[:, nt, :], ps[:], bb2_sb[:])

            # ================================================================
            # segment-mean pooling + AllReduce
            # ================================================================
            with tc.tile_pool(name="segp", bufs=1) as sgp, \
                 tc.tile_pool(name="psx", bufs=1, space="PSUM") as psx:
                P1_sb = sgp.tile([128, c_.NB, c_.SEGP], F16)
                nc.sync.dma_start(P1_sb[:], P1_d[:, :, :])
                P3_sb = sgp.tile([128, c_.NB, c_.SEGP], F16)
                nc.sync.dma_start(P3_sb[:], P3_d[:, :, :])
                x1a = psx.tile([128, DIM], F32, tag="x1a", bufs=1)
                x1b = psx.tile([64, DIM], F32, tag="x1b", bufs=1)
                x3a = psx.tile([128, DIM], F32, tag="x3a", bufs=1)
                x3b = psx.tile([64, DIM], F32, tag="x3b", bufs=1)
                for nt in range(c_.NB):
                    rhs = h3_sb[:, nt, :]
                    s0, s1 = (nt == 0), (nt == c_.NB - 1)
                    nc.tensor.matmul(x1a[:], P1_sb[:, nt, 0:128], rhs, start=s0, stop=s1)
                    nc.tensor.matmul(x1b[:], P1_sb[:, nt, 128:192], rhs, start=s0, stop=s1)
                    nc.tensor.matmul(x3a[:], P3_sb[:, nt, 0:128], rhs, start=s0, stop=s1)
                    nc.tensor.matmul(x3b[:], P3_sb[:, nt, 128:192], rhs, start=s0, stop=s1)
                xc1 = sgp.tile([128, 3 * DIM], F32)
                xc2 = sgp.tile([64, 3 * DIM], F32)
                nc.vector.tensor_copy(xc1[:, 0:DIM], x1a[:])
                nc.vector.tensor_copy(xc1[:, DIM : 2 * DIM], x3a[:])
                nc.vector.tensor_copy(xc1[:, 2 * DIM : 3 * DIM], x3a[:])
                nc.vector.tensor_copy(xc2[:, 0:DIM], x1b[:])
                nc.vector.tensor_copy(xc2[:, DIM : 2 * DIM], x3b[:])
                nc.vector.tensor_copy(xc2[:, 2 * DIM : 3 * DIM], x3b[:])
                nc.sync.dma_start(xcat_dram[0:128, :], xc1[:])
                nc.sync.dma_start(xcat_dram[128 : c_.SEGP, :], xc2[:])
            nc.gpsimd.collective_compute(
                "AllReduce",
                ALU.add,
                replica_groups=[groups_all],
                ins=[xcat_dram[:, :]],
                outs=[xcat_red[:, :]],
            )

            # ================================================================
            # final MLP (fp32) + log_softmax, replicated
            # ================================================================
            with tc.tile_pool(name="fin", bufs=1) as fp, \
                 tc.tile_pool(name="psf", bufs=2, space="PSUM") as psf:
                ident_sb = fp.tile([128, 128], F32)
                nc.sync.dma_start(ident_sb[:], ident_d[:, :])
                xr1 = fp.tile([128, 3 * DIM], F32)
                nc.sync.dma_start(xr1[:], xcat_red[0:128, :])
                xr2 = fp.tile([64, 3 * DIM], F32)
                nc.sync.dma_start(xr2[:], xcat_red[128 : c_.SEGP, :])
                xcT = fp.tile([128, c_.KCAT, c_.SEGP], F32)
                for kc in range(c_.KCAT):
                    pt = psf.tile([128, 128], F32, tag="ptr")
                    nc.tensor.transpose(pt[:], xr1[:, kc * 128 : (kc + 1) * 128], ident_sb[:])
                    nc.vector.tensor_copy(xcT[:, kc, 0:128], pt[:])
                    pt2 = psf.tile([128, 64], F32, tag="ptr2", bufs=1)
                    nc.tensor.transpose(
                        pt2[:], xr2[:, kc * 128 : (kc + 1) * 128], ident_sb[0:64, 0:64]
                    )
                    nc.vector.tensor_copy(xcT[:, kc, 128 : c_.SEGP], pt2[:])
                mw1_sb = fp.tile([128, c_.KCAT, DIM], F32)
                nc.sync.dma_start(mw1_sb[:], mw1_d.ap().rearrange("(k p) d -> p k d", p=128))
                mb1_sb = fp.tile([128, DIM], F32)
                nc.sync.dma_start(mb1_sb[:], mb1_d[:, :])
                y1a = psf.tile([128, DIM], F32, tag="y1a", bufs=1)
                y1b = psf.tile([64, DIM], F32, tag="y1b", bufs=1)
                for kc in range(c_.KCAT):
                    s0, s1 = (kc == 0), (kc == c_.KCAT - 1)
                    nc.tensor.matmul(y1a[:], xcT[:, kc, 0:128], mw1_sb[:, kc, :], start=s0, stop=s1)
                    nc.tensor.matmul(y1b[:], xcT[:, kc, 128:192], mw1_sb[:, kc, :], start=s0, stop=s1)
                y1s1 = fp.tile([128, DIM], F32)
                nc.vector.tensor_add(y1s1[:], y1a[:], mb1_sb[:])
                nc.vector.tensor_scalar_max(y1s1[:], y1s1[:], 0.0)
                y1s2 = fp.tile([64, DIM], F32)
                nc.vector.tensor_add(y1s2[:], y1b[:], mb1_sb[0:64, :])
                nc.vector.tensor_scalar_max(y1s2[:], y1s2[:], 0.0)
                y1T = fp.tile([128, c_.KD, c_.SEGP], F32)
                for kc in range(c_.KD):
                    pt = psf.tile([128, 128], F32, tag="ptr")
                    nc.tensor.transpose(pt[:], y1s1[:, kc * 128 : (kc + 1) * 128], ident_sb[:])
                    nc.vector.tensor_copy(y1T[:, kc, 0:128], pt[:])
                    pt2 = psf.tile([128, 64], F32, tag="ptr2", bufs=1)
                    nc.tensor.transpose(
                        pt2[:], y1s2[:, kc * 128 : (kc + 1) * 128], ident_sb[0:64, 0:64]
                    )
                    nc.vector.tensor_copy(y1T[:, kc, 128 : c_.SEGP], pt2[:])
                mw2_sb = fp.tile([128, c_.KD, c_.C], F32)
                nc.sync.dma_start(mw2_sb[:], mw2_d.ap().rearrange("(k p) d -> p k d", p=128))
                mb2_sb = fp.tile([128, c_.C], F32)
                nc.sync.dma_start(mb2_sb[:], mb2_d[:, :])
                la = psf.tile([128, c_.C], F32, tag="la", bufs=1)
                lb = psf.tile([64, c_.C], F32, tag="lb", bufs=1)
                for kc in range(c_.KD):
                    s0, s1 = (kc == 0), (kc == c_.KD - 1)
                    nc.tensor.matmul(la[:], y1T[:, kc, 0:128], mw2_sb[:, kc, :], start=s0, stop=s1)
                    nc.tensor.matmul(lb[:], y1T[:, kc, 128:192], mw2_sb[:, kc, :], start=s0, stop=s1)
                outs = []
                for ps_, bias_, np_ in ((la[:], mb2_sb[:], 128), (lb[:], mb2_sb[0:64, :], 64)):
                    lg = fp.tile([np_, c_.C], F32, tag=f"lg{np_}", name=f"lg{np_}")
                    nc.vector.tensor_add(lg[:], ps_, bias_)
                    mx = fp.tile([np_, 1], F32, tag=f"mx{np_}")
                    nc.vector.tensor_reduce(mx[:], lg[:], AX.X, ALU.max)
                    tt = fp.tile([np_, c_.C], F32, tag=f"tt{np_}")
                    nc.vector.tensor_scalar(tt[:], lg[:], mx[:], None, ALU.subtract)
                    ex = fp.tile([np_, c_.C], F32, tag=f"ex{np_}")
                    nc.scalar.activation(ex[:], tt[:], AF.Exp)
                    sm = fp.tile([np_, 1], F32, tag=f"sm{np_}")
                    nc.vector.tensor_reduce(sm[:], ex[:], AX.X, ALU.add)
                    ln = fp.tile([np_, 1], F32, tag=f"ln{np_}")
                    nc.scalar.activation(ln[:], sm[:], AF.Ln)
                    lp = fp.tile([np_, c_.C], F32, tag=f"lp{np_}")
                    nc.vector.tensor_scalar(lp[:], tt[:], ln[:], None, ALU.subtract)
                    outs.append(lp)
                nc.sync.dma_start(out_d[0:128, :], outs[0][:])
                nc.sync.dma_start(out_d[128 : c_.NORIG, :], outs[1][0 : c_.NORIG - 128, :])

    nc.compile()
    return nc


# --------------------------------------------------------------------------
# entry point
# --------------------------------------------------------------------------

def kernel(**inputs):
    meta, in_maps = prep(inputs, DEFAULT_CFG)
    nc = build(meta, DEFAULT_CFG)
    res = bass_utils.run_bass_kernel_spmd(nc, in_maps, core_ids=list(range(NCORES)))
    out = np.asarray(res.results[0]["out"], np.float32)
    return out


# revision 22
# speedup vs baseline: 1.0781x; 1.0097x over previous
"""Trainium2 Bass kernel for a 3-relation GCN (gnn_message_passing).

Strategy (8 NeuronCores, SPMD):
  - Node dim (20000 -> padded 20480) sharded 2560/core.
  - Layer GCN matmuls computed locally; per-relation [2560,256] fp16 shards
    AllGathered to a full node-feature table in DRAM.
  - Edge aggregation: edges partitioned by destination (host side), sorted,
    chunked into 128-edge groups per 64-dst window.  h rows are fetched with
    gpsimd dma_gather; a host-built one-hot scatter matrix S (carrying the
    symmetric GCN norms + bias edges) turns the scatter-add into PE matmuls
    accumulating in PSUM.
  - MLPs run from DMA-transposed (xbar) activations so the contraction dim
    lands on partitions.
  - Segment-mean pooling is a one-hot (1/cnt) matmul; the [192,768] fp32
    partials are AllReduced; the tiny final MLP + log_softmax is replicated.
All device math fp16 with fp32 PSUM accumulation; final stage fp32.
"""

import sys

import numpy as np

sys.path.insert(0, "/opt/trn_rl_repo")

import concourse.bacc as bacc  # noqa: E402
import concourse.bass as bass  # noqa: E402
import concourse.mybir as mybir  # noqa: E402
import concourse.tile as tile  # noqa: E402
from concourse import bass_utils  # noqa: E402

F16 = mybir.dt.float16
F32 = mybir.dt.float32
I16 = mybir.dt.int16
AF = mybir.ActivationFunctionType
ALU = mybir.AluOpType
AX = mybir.AxisListType

NCORES = 8


class Cfg:
    def __init__(self, n=20000, e=320000, f_in=5189, dim=256, norig=183, ncls=7,
                 npad=20480, fpad=5248, winw=64, nsup=10):
        self.N = n
        self.E = e
        self.F_IN = f_in
        self.DIM = dim
        self.NORIG = norig
        self.C = ncls
        self.NPAD = npad
        self.SH = npad // NCORES          # nodes per core
        self.NB = self.SH // 128          # 128-dst blocks per core
        self.WINW = winw                  # dst window width for scatter matmuls
        self.NWIN = self.SH // winw
        self.WPB = 128 // winw            # windows per block
        self.FPAD = fpad
        self.KC1 = fpad // 128
        self.NSUP = nsup                  # x stripes
        self.SUPW = self.SH // nsup
        self.BIAS_ROW = npad
        self.HROWS = npad + 1
        self.SEGP = 192                   # padded NORIG
        self.KCAT = 3 * dim // 128        # 6
        self.KD = dim // 128              # 2
        assert self.SH % 128 == 0 and self.SUPW % 128 == 0 and fpad % 128 == 0


DEFAULT_CFG = Cfg()


# --------------------------------------------------------------------------
# host-side preprocessing
# --------------------------------------------------------------------------

def _wrap_idx(idx, cfg):
    """dma_gather index layout: pos i -> partition i%16, col i//16; the 16-row
    block replicated across the 8 GPSIMD cores (128 partitions)."""
    tg8 = idx.shape[0] // 16
    w = idx.astype(np.int16).reshape(tg8, 16).T            # [16, tg*8]
    return np.ascontiguousarray(np.tile(w, (8, 1)))        # [128, tg*8]


def prep(inputs, cfg=DEFAULT_CFG):
    c_ = cfg
    f16 = np.float16
    x = np.asarray(inputs["x"], np.float32)
    w1 = np.asarray(inputs["w1"], np.float32)
    b1 = np.asarray(inputs["b1"], np.float32)
    w2 = np.asarray(inputs["w2"], np.float32)
    b2 = np.asarray(inputs["b2"], np.float32)
    eis = [np.asarray(inputs[f"edge_index_{k}"], np.int64) for k in (1, 2, 3)]
    idx1 = np.asarray(inputs["index_1"], np.int64)
    idx3 = np.asarray(inputs["index_3"], np.int64)

    # ---- per-relation edge prep: self loops, degrees, norms, dst partition
    Gw = np.zeros((3, c_.NWIN), np.int64)   # groups per (relative) window, max over cores
    core_edges = [[None] * 3 for _ in range(NCORES)]
    loop = np.arange(c_.N)
    for r in range(3):
        src = np.concatenate([eis[r][0], loop])
        dst = np.concatenate([eis[r][1], loop])
        deg = np.bincount(dst, minlength=c_.N).astype(np.float64)
        dinv = 1.0 / np.sqrt(deg)
        norm = (dinv[src] * dinv[dst]).astype(np.float32)
        for c in range(NCORES):
            lo = c * c_.SH
            m = (dst >= lo) & (dst < lo + c_.SH)
            es, ed, en = src[m], dst[m] - lo, norm[m]
            nreal = int(min(max(c_.N - lo, 0), c_.SH))     # real dst rows in shard
            es = np.concatenate([es, np.full(nreal, c_.BIAS_ROW, np.int64)])
            ed = np.concatenate([ed, np.arange(nreal)])
            en = np.concatenate([en, np.ones(nreal, np.float32)])
            o = np.argsort(ed, kind="stable")
            es, ed, en = es[o], ed[o], en[o]
            core_edges[c][r] = (es, ed, en)
            cnts = np.bincount(ed // c_.WINW, minlength=c_.NWIN)
            Gw[r] = np.maximum(Gw[r], (cnts + 127) // 128)
    Gw = np.maximum(Gw, 1)  # every window needs >=1 group so its PSUM is written
    TG = Gw.sum(axis=1)

    # ---- pooling one-hot (1/cnt) matrices, global counts
    cnt1 = np.bincount(idx1, minlength=c_.NORIG).astype(np.float64)
    cnt3 = np.bincount(idx3, minlength=c_.NORIG).astype(np.float64)
    r1 = (1.0 / np.maximum(cnt1, 1.0)).astype(np.float32)
    r3 = (1.0 / np.maximum(cnt3, 1.0)).astype(np.float32)
    P1 = np.zeros((NCORES, 128, c_.NB, c_.SEGP), f16)
    P3 = np.zeros((NCORES, 128, c_.NB, c_.SEGP), f16)
    nn = np.arange(c_.N)
    cc, ll = nn // c_.SH, nn % c_.SH
    P1[cc, ll % 128, ll // 128, idx1] = r1[idx1].astype(f16)
    P3[cc, ll % 128, ll // 128, idx3] = r3[idx3].astype(f16)

    # ---- shared (replicated) tensors
    w1h = np.zeros((3, c_.FPAD, c_.DIM), f16)
    w1h[:, : c_.F_IN] = w1.astype(f16)
    shared = {
        "w1h": w1h,
        "w2h": w2.astype(f16),
        "b1row": b1.astype(f16).reshape(1, -1),
        "b2row": b2.astype(f16).reshape(1, -1),
        "m1w1h": np.asarray(inputs["m1w1"], np.float32).astype(f16),
        "m1w2h": np.asarray(inputs["m1w2"], np.float32).astype(f16),
        "m2w1h": np.asarray(inputs["m2w1"], np.float32).astype(f16),
        "m2w2h": np.asarray(inputs["m2w2"], np.float32).astype(f16),
        "m1b1bc": np.tile(np.asarray(inputs["m1b1"], np.float32), (128, 1)),
        "m1b2bc": np.tile(np.asarray(inputs["m1b2"], np.float32), (128, 1)),
        "m2b1bc": np.tile(np.asarray(inputs["m2b1"], np.float32), (128, 1)),
        "m2b2bc": np.tile(np.asarray(inputs["m2b2"], np.float32), (128, 1)),
        "mw1f": np.asarray(inputs["mw1"], np.float32),
        "mw2f": np.asarray(inputs["mw2"], np.float32),
        "mb1bc": np.tile(np.asarray(inputs["mb1"], np.float32), (128, 1)),
        "mb2bc": np.tile(np.asarray(inputs["mb2"], np.float32), (128, 1)),
        "ident": np.eye(128, dtype=np.float32),
    }

    # ---- per-core tensors
    in_maps = []
    for c in range(NCORES):
        d = dict(shared)
        lo = c * c_.SH
        nreal = int(min(max(c_.N - lo, 0), c_.SH))
        xs = np.zeros((c_.SH, c_.FPAD), f16)
        xs[:nreal, : c_.F_IN] = x[lo : lo + nreal].astype(f16)
        xT = np.ascontiguousarray(
            xs.T.reshape(c_.FPAD, c_.NSUP, c_.SUPW).transpose(1, 0, 2)
        )
        d["xT"] = xT
        for r in range(3):
            es, ed, en = core_edges[c][r]
            win = ed // c_.WINW
            tg = int(TG[r])
            sp = np.full(tg * 128, c_.BIAS_ROW, np.int64)
            dl = np.zeros(tg * 128, np.int64)
            nm = np.zeros(tg * 128, np.float32)
            st = np.searchsorted(win, np.arange(c_.NWIN), side="left")
            en_ = np.searchsorted(win, np.arange(c_.NWIN), side="right")
            g0 = 0
            for w in range(c_.NWIN):
                cnt = int(en_[w] - st[w])
                sl = slice(g0 * 128, g0 * 128 + cnt)
                sp[sl] = es[st[w] : en_[w]]
                dl[sl] = ed[st[w] : en_[w]] - w * c_.WINW
                nm[sl] = en[st[w] : en_[w]]
                g0 += int(Gw[r][w])
            S = np.zeros((tg * 128, c_.WINW), f16)
            S[np.arange(tg * 128), dl] = nm.astype(f16)  # pad rows write 0.0
            d[f"S_{r}"] = np.ascontiguousarray(
                S.reshape(tg, 128, c_.WINW).transpose(1, 0, 2).reshape(128, tg * c_.WINW)
            )
            d[f"gidx_{r}"] = _wrap_idx(sp, c_)
        d["P1"] = np.ascontiguousarray(P1[c])
        d["P3"] = np.ascontiguousarray(P3[c])
        in_maps.append(d)

    meta = {"Gw": Gw.astype(int), "TG": [int(t) for t in TG]}
    return meta, in_maps


# --------------------------------------------------------------------------
# device program
# --------------------------------------------------------------------------

def build(meta, cfg=DEFAULT_CFG, enable_asserts=False):
    c_ = cfg
    Gw, TG = meta["Gw"], meta["TG"]
    nc = bacc.Bacc(
        "TRN2",
        target_bir_lowering=False,
        debug=False,
        enable_asserts=enable_asserts,
        num_devices=NCORES,
        num_swdge_queues=4,
    )

    # ---------------- DRAM I/O
    xT_d = nc.dram_tensor("xT", [c_.NSUP, c_.FPAD, c_.SUPW], F16, kind="ExternalInput")
    w1_d = nc.dram_tensor("w1h", [3, c_.FPAD, c_.DIM], F16, kind="ExternalInput")
    w2_d = nc.dram_tensor("w2h", [3, c_.DIM, c_.DIM], F16, kind="ExternalInput")
    b1_d = nc.dram_tensor("b1row", [1, 3 * c_.DIM], F16, kind="ExternalInput")
    b2_d = nc.dram_tensor("b2row", [1, 3 * c_.DIM], F16, kind="ExternalInput")
    m1w1_d = nc.dram_tensor("m1w1h", [3 * c_.DIM, c_.DIM], F16, kind="ExternalInput")
    m1w2_d = nc.dram_tensor("m1w2h", [c_.DIM, c_.DIM], F16, kind="ExternalInput")
    m2w1_d = nc.dram_tensor("m2w1h", [3 * c_.DIM, c_.DIM], F16, kind="ExternalInput")
    m2w2_d = nc.dram_tensor("m2w2h", [c_.DIM, c_.DIM], F16, kind="ExternalInput")
    m1b1_d = nc.dram_tensor("m1b1bc", [128, c_.DIM], F32, kind="ExternalInput")
    m1b2_d = nc.dram_tensor("m1b2bc", [128, c_.DIM], F32, kind="ExternalInput")
    m2b1_d = nc.dram_tensor("m2b1bc", [128, c_.DIM], F32, kind="ExternalInput")
    m2b2_d = nc.dram_tensor("m2b2bc", [128, c_.DIM], F32, kind="ExternalInput")
    mw1_d = nc.dram_tensor("mw1f", [3 * c_.DIM, c_.DIM], F32, kind="ExternalInput")
    mw2_d = nc.dram_tensor("mw2f", [c_.DIM, c_.C], F32, kind="ExternalInput")
    mb1_d = nc.dram_tensor("mb1bc", [128, c_.DIM], F32, kind="ExternalInput")
    mb2_d = nc.dram_tensor("mb2bc", [128, c_.C], F32, kind="ExternalInput")
    ident_d = nc.dram_tensor("ident", [128, 128], F32, kind="ExternalInput")
    S_d = [
        nc.dram_tensor(f"S_{r}", [128, TG[r] * c_.WINW], F16, kind="ExternalInput")
        for r in range(3)
    ]
    gidx_d = [
        nc.dram_tensor(f"gidx_{r}", [128, TG[r] * 8], I16, kind="ExternalInput")
        for r in range(3)
    ]
    P1_d = nc.dram_tensor("P1", [128, c_.NB, c_.SEGP], F16, kind="ExternalInput")
    P3_d = nc.dram_tensor("P3", [128, c_.NB, c_.SEGP], F16, kind="ExternalInput")
    out_d = nc.dram_tensor("out", [c_.NORIG, c_.C], F32, kind="ExternalOutput")

    groups_all = list(range(NCORES))
    DIM = c_.DIM

    with tile.TileContext(nc) as tc:
        with tc.tile_pool(name="dram", bufs=1, space="DRAM") as dpool, \
             tc.tile_pool(name="consts", bufs=1) as cpool:
            # DRAM intermediates
            h_loc = [[dpool.tile([c_.SH, DIM], F16, name=f"hloc{l}_{r}")
                      for r in range(3)] for l in range(2)]
            h_full = [[dpool.tile([c_.HROWS, DIM], F16, name=f"hfull{l}_{r}")
                       for r in range(3)] for l in range(2)]
            hcat_dram = [dpool.tile([c_.SH, 3 * DIM], F16, name=f"hcat{l}")
                         for l in range(2)]
            z1_dram = [dpool.tile([c_.SH, DIM], F16, name=f"z1d{l}") for l in range(2)]
            h2_dram = dpool.tile([c_.SH, DIM], F16, name="h2d")
            xcat_dram = dpool.tile([c_.SEGP, 3 * DIM], F32, name="xcatd")
            xcat_red = dpool.tile([c_.SEGP, 3 * DIM], F32, name="xcatr")

            # persistent small consts
            b_sb = cpool.tile([1, 3 * DIM * 2], F16)       # b1|b2 rows
            nc.sync.dma_start(b_sb[:, 0 : 3 * DIM], b1_d[:, :])
            nc.sync.dma_start(b_sb[:, 3 * DIM : 6 * DIM], b2_d[:, :])
            idx_sb = []
            for r in range(3):
                t = cpool.tile([128, TG[r] * 8], I16, name=f"idx{r}")
                nc.sync.dma_start(t[:], gidx_d[r][:, :])
                idx_sb.append(t)

            # ----------------------------------------------------------------
            def allgather(l, r, bsrc):
                nc.gpsimd.collective_compute(
                    "AllGather",
                    ALU.bypass,
                    replica_groups=[groups_all],
                    ins=[h_loc[l][r][:, :]],
                    outs=[h_full[l][r][0 : c_.NPAD, :]],
                )
                # bias row (virtual node) for this layer/relation
                nc.sync.dma_start(
                    h_full[l][r][c_.BIAS_ROW : c_.BIAS_ROW + 1, :],
                    bsrc[:, r * DIM : (r + 1) * DIM],
                )

            def aggregate(l, r, gpool, spool, psp, stp):
                """scatter-add via one-hot matmuls; writes relu result to hcat."""
                gb = 0
                for b in range(c_.NB):
                    qn = b % 4
                    nws = [int(Gw[r][c_.WPB * b + k]) for k in range(c_.WPB)]
                    Gb = sum(nws)
                    gt = gpool.tile([128, Gb, DIM], F16, tag="gath")
                    nc.gpsimd.dma_gather(
                        gt[:],
                        h_full[l][r][:, :],
                        idx_sb[r][:, gb * 8 : (gb + Gb) * 8],
                        Gb * 128,
                        Gb * 128,
                        DIM,
                        # single_packet caps at 64 descs/engine = 1024 idxs
                        single_packet=(Gb * 128 <= 1024),
                        queue_num=qn,
                    )
                    s_sb = spool.tile([128, Gb, c_.WINW], F16, tag="sgt")
                    nc.sync.dma_start(
                        s_sb[:], S_d[r][:, gb * c_.WINW : (gb + Gb) * c_.WINW]
                    )
                    ps = psp.tile([128, DIM], F32, tag="psagg")
                    g = 0
                    for k in range(c_.WPB):
                        w0 = k * c_.WINW
                        for j in range(nws[k]):
                            nc.tensor.matmul(
                                ps[w0 : w0 + c_.WINW, :],
                                s_sb[:, g, :],
                                gt[:, g, :],
                                start=(j == 0),
                                stop=(j == nws[k] - 1),
                            )
                            g += 1
                    st = stp.tile([128, DIM], F16, tag="aggst")
                    nc.vector.tensor_scalar_max(st[:], ps[:], 0.0)  # relu + cast
                    nc.sync.dma_start(
                        hcat_dram[l][b * 128 : (b + 1) * 128, r * DIM : (r + 1) * DIM],
                        st[:],
                    )
                    gb += Gb

            # ================================================================
            # Layer 1: x @ W1_r  (lhsT = xT stripes)
            # ================================================================
            with tc.tile_pool(name="w1p", bufs=1) as w1p, \
                 tc.tile_pool(name="xsp", bufs=2) as xsp, \
                 tc.tile_pool(name="psa", bufs=2, space="PSUM") as psa, \
                 tc.tile_pool(name="hst", bufs=6) as hstp:
                w1_sb = w1p.tile([128, 3 * c_.KC1, DIM], F16)
                for r in range(3):
                    nc.sync.dma_start(
                        w1_sb[:, r * c_.KC1 : (r + 1) * c_.KC1, :],
                        w1_d[r].rearrange("(k p) d -> p k d", p=128),
                    )
                ntps = c_.SUPW // 128
                for ns in range(c_.NSUP):
                    xs_sb = xsp.tile([128, c_.KC1, c_.SUPW], F16, tag="xs")
                    nc.sync.dma_start(
                        xs_sb[:], xT_d[ns].rearrange("(k p) w -> p k w", p=128)
                    )
                    for ntl in range(ntps):
                        nt = ns * ntps + ntl
                        pss = [psa.tile([128, DIM], F32, tag=f"psa{r}", name=f"psa{r}")
                               for r in range(3)]
                        for kc in range(c_.KC1):
                            lhsT = xs_sb[:, kc, ntl * 128 : (ntl + 1) * 128]
                            for r in range(3):
                                nc.tensor.matmul(
                                    pss[r][:],
                                    lhsT,
                                    w1_sb[:, r * c_.KC1 + kc, :],
                                    start=(kc == 0),
                                    stop=(kc == c_.KC1 - 1),
                                )
                        for r in range(3):
                            st = hstp.tile([128, DIM], F16, tag="hstage")
                            nc.vector.tensor_copy(st[:], pss[r][:])
                            nc.sync.dma_start(
                                h_loc[0][r][nt * 128 : (nt + 1) * 128, :], st[:]
                            )

            # ================================================================
            # per-layer: allgather -> aggregate -> MLP
            # ================================================================
            for l in range(2):
                bsrc = b_sb[:, 0 : 3 * DIM] if l == 0 else b_sb[:, 3 * DIM : 6 * DIM]
                for r in range(3):
                    allgather(l, r, bsrc)
                with tc.tile_pool(name=f"gp{l}", bufs=8) as gpool, \
                     tc.tile_pool(name=f"sp{l}", bufs=8) as spool, \
                     tc.tile_pool(name=f"pc{l}", bufs=4, space="PSUM") as psp, \
                     tc.tile_pool(name=f"st{l}", bufs=6) as stp:
                    for r in range(3):
                        aggregate(l, r, gpool, spool, psp, stp)

                # MLP: hcat -> z1 (relu) -> h2
                w1name = m1w1_d if l == 0 else m2w1_d
                w2name = m1w2_d if l == 0 else m2w2_d
                bb1 = m1b1_d if l == 0 else m2b1_d
                bb2 = m1b2_d if l == 0 else m2b2_d
                with tc.tile_pool(name=f"mlp{l}", bufs=1) as mp, \
                     tc.tile_pool(name=f"psm{l}", bufs=2, space="PSUM") as psm, \
                     tc.tile_pool(name=f"mst{l}", bufs=4) as mst:
                    hcatT = mp.tile([128, c_.KCAT, c_.SH], F16)
                    for kt in range(c_.KCAT):
                        nc.sync.dma_start_transpose(
                            hcatT[:, kt, :], hcat_dram[l][:, kt * 128 : (kt + 1) * 128]
                        )
                    mw1_sb = mp.tile([128, c_.KCAT, DIM], F16)
                    nc.sync.dma_start(mw1_sb[:], w1name.ap().rearrange("(k p) d -> p k d", p=128))
                    mw2_sb = mp.tile([128, c_.KD, DIM], F16)
                    nc.sync.dma_start(mw2_sb[:], w2name.ap().rearrange("(k p) d -> p k d", p=128))
                    bb1_sb = mp.tile([128, DIM], F32)
                    nc.sync.dma_start(bb1_sb[:], bb1[:, :])
                    bb2_sb = mp.tile([128, DIM], F32)
                    nc.sync.dma_start(bb2_sb[:], bb2[:, :])

                    for nt in range(c_.NB):
                        ps = psm.tile([128, DIM], F32, tag="psz1")
                        for kc in range(c_.KCAT):
                            nc.tensor.matmul(
                                ps[:],
                                hcatT[:, kc, nt * 128 : (nt + 1) * 128],
                                mw1_sb[:, kc, :],
                                start=(kc == 0),
                                stop=(kc == c_.KCAT - 1),
                            )
                        tmp = mst.tile([128, DIM], F32, tag="ztmp")
                        nc.vector.tensor_add(tmp[:], ps[:], bb1_sb[:])
                        z1st = mst.tile([128, DIM], F16, tag="z1st")
                        nc.vector.tensor_scalar_max(z1st[:], tmp[:], 0.0)
                        nc.sync.dma_start(
                            z1_dram[l][nt * 128 : (nt + 1) * 128, :], z1st[:]
                        )
                    z1T = mp.tile([128, c_.KD, c_.SH], F16)
                    for kt in range(c_.KD):
                        nc.sync.dma_start_transpose(
                            z1T[:, kt, :], z1_dram[l][:, kt * 128 : (kt + 1) * 128]
                        )
                    if l == 0:
                        for nt in range(c_.NB):
                            ps = psm.tile([128, DIM], F32, tag="psz2")
                            for kc in range(c_.KD):
                                nc.tensor.matmul(
                                    ps[:],
                                    z1T[:, kc, nt * 128 : (nt + 1) * 128],
                                    mw2_sb[:, kc, :],
                                    start=(kc == 0),
                                    stop=(kc == c_.KD - 1),
                                )
                            h2st = mst.tile([128, DIM], F16, tag="h2st")
                            nc.vector.tensor_add(h2st[:], ps[:], bb2_sb[:])
                            nc.sync.dma_start(
                                h2_dram[nt * 128 : (nt + 1) * 128, :], h2st[:]
                            )
                        # layer-2 GCN matmuls: h2 @ W2_r
                        h2T = mp.tile([128, c_.KD, c_.SH], F16)
                        for kt in range(c_.KD):
                            nc.sync.dma_start_transpose(
                                h2T[:, kt, :], h2_dram[:, kt * 128 : (kt + 1) * 128]
                            )
                        w2_sb = mp.tile([128, 3 * c_.KD, DIM], F16)
                        for r in range(3):
                            nc.sync.dma_start(
                                w2_sb[:, r * c_.KD : (r + 1) * c_.KD, :],
                                w2_d[r].rearrange("(k p) d -> p k d", p=128),
                            )
                        for nt in range(c_.NB):
                            pss = [psm.tile([128, DIM], F32, tag=f"psg{r}",
                                            name=f"psg{r}", bufs=1) for r in range(3)]
                            for kc in range(c_.KD):
                                lhsT = h2T[:, kc, nt * 128 : (nt + 1) * 128]
                                for r in range(3):
                                    nc.tensor.matmul(
                                        pss[r][:],
                                        lhsT,
                                        w2_sb[:, r * c_.KD + kc, :],
                                        start=(kc == 0),
                                        stop=(kc == c_.KD - 1),
                                    )
                            for r in range(3):
                                st = mst.tile([128, DIM], F16, tag="hstage2")
                                nc.vector.tensor_copy(st[:], pss[r][:])
                                nc.sync.dma_start(
                                    h_loc[1][r][nt * 128 : (nt + 1) * 128, :], st[:]
                                )
                    else:
                        # final node features h3 (kept in SBUF)
                        h3_sb = cpool.tile([128, c_.NB, DIM], F16)
                        for nt in range(c_.NB):
                            ps = psm.tile([128, DIM], F32, tag="psz2")
                            for kc in range(c_.KD):
                                nc.tensor.matmul(
                                    ps[:],
                                    z1T[:, kc, nt * 128 : (nt + 1) * 128],
                                    mw2_sb[:, kc, :],
                                    start=(kc == 0),
                                    stop=(kc == c_.KD - 1),
                                )
                            nc.vector.tensor_add(h3_sb[:, nt, :], ps[:], bb2_sb[:])

            # ================================================================
            # segment-mean pooling + AllReduce
            # ================================================================
            with tc.tile_pool(name="segp", bufs=1) as sgp, \
                 tc.tile_pool(name="psx", bufs=1, space="PSUM") as psx:
                P1_sb = sgp.tile([128, c_.NB, c_.SEGP], F16)
                nc.sync.dma_start(P1_sb[:], P1_d[:, :, :])
                P3_sb = sgp.tile([128, c_.NB, c_.SEGP], F16)
                nc.sync.dma_start(P3_sb[:], P3_d[:, :, :])
                x1a = psx.tile([128, DIM], F32, tag="x1a", bufs=1)
                x1b = psx.tile([64, DIM], F32, tag="x1b", bufs=1)
                x3a = psx.tile([128, DIM], F32, tag="x3a", bufs=1)
                x3b = psx.tile([64, DIM], F32, tag="x3b", bufs=1)
                for nt in range(c_.NB):
                    rhs = h3_sb[:, nt, :]
                    s0, s1 = (nt == 0), (nt == c_.NB - 1)
                    nc.tensor.matmul(x1a[:], P1_sb[:, nt, 0:128], rhs, start=s0, stop=s1)
                    nc.tensor.matmul(x1b[:], P1_sb[:, nt, 128:192], rhs, start=s0, stop=s1)
                    nc.tensor.matmul(x3a[:], P3_sb[:, nt, 0:128], rhs, start=s0, stop=s1)
                    nc.tensor.matmul(x3b[:], P3_sb[:, nt, 128:192], rhs, start=s0, stop=s1)
                xc1 = sgp.tile([128, 3 * DIM], F32)
                xc2 = sgp.tile([64, 3 * DIM], F32)
                nc.vector.tensor_copy(xc1[:, 0:DIM], x1a[:])
                nc.vector.tensor_copy(xc1[:, DIM : 2 * DIM], x3a[:])
                nc.vector.tensor_copy(xc1[:, 2 * DIM : 3 * DIM], x3a[:])
                nc.vector.tensor_copy(xc2[:, 0:DIM], x1b[:])
                nc.vector.tensor_copy(xc2[:, DIM : 2 * DIM], x3b[:])
                nc.vector.tensor_copy(xc2[:, 2 * DIM : 3 * DIM], x3b[:])
                nc.sync.dma_start(xcat_dram[0:128, :], xc1[:])
                nc.sync.dma_start(xcat_dram[128 : c_.SEGP, :], xc2[:])
            nc.gpsimd.collective_compute(
                "AllReduce",
                ALU.add,
                replica_groups=[groups_all],
                ins=[xcat_dram[:, :]],
                outs=[xcat_red[:, :]],
            )

            # ================================================================
            # final MLP (fp32) + log_softmax, replicated
            # ================================================================
            with tc.tile_pool(name="fin", bufs=1) as fp, \
                 tc.tile_pool(name="psf", bufs=2, space="PSUM") as psf:
                ident_sb = fp.tile([128, 128], F32)
                nc.sync.dma_start(ident_sb[:], ident_d[:, :])
                xr1 = fp.tile([128, 3 * DIM], F32)
                nc.sync.dma_start(xr1[:], xcat_red[0:128, :])
                xr2 = fp.tile([64, 3 * DIM], F32)
                nc.sync.dma_start(xr2[:], xcat_red[128 : c_.SEGP, :])
                xcT = fp.tile([128, c_.KCAT, c_.SEGP], F32)
                for kc in range(c_.KCAT):
                    pt = psf.tile([128, 128], F32, tag="ptr")
                    nc.tensor.transpose(pt[:], xr1[:, kc * 128 : (kc + 1) * 128], ident_sb[:])
                    nc.vector.tensor_copy(xcT[:, kc, 0:128], pt[:])
                    pt2 = psf.tile([128, 64], F32, tag="ptr2", bufs=1)
                    nc.tensor.transpose(
                        pt2[:], xr2[:, kc * 128 : (kc + 1) * 128], ident_sb[0:64, 0:64]
                    )
                    nc.vector.tensor_copy(xcT[:, kc, 128 : c_.SEGP], pt2[:])
                mw1_sb = fp.tile([128, c_.KCAT, DIM], F32)
                nc.sync.dma_start(mw1_sb[:], mw1_d.ap().rearrange("(k p) d -> p k d", p=128))
                mb1_sb = fp.tile([128, DIM], F32)
                nc.sync.dma_start(mb1_sb[:], mb1_d[:, :])
                y1a = psf.tile([128, DIM], F32, tag="y1a", bufs=1)
                y1b = psf.tile([64, DIM], F32, tag="y1b", bufs=1)
                for kc in range(c_.KCAT):
                    s0, s1 = (kc == 0), (kc == c_.KCAT - 1)
                    nc.tensor.matmul(y1a[:], xcT[:, kc, 0:128], mw1_sb[:, kc, :], start=s0, stop=s1)
                    nc.tensor.matmul(y1b[:], xcT[:, kc, 128:192], mw1_sb[:, kc, :], start=s0, stop=s1)
                y1s1 = fp.tile([128, DIM], F32)
                nc.vector.tensor_add(y1s1[:], y1a[:], mb1_sb[:])
                nc.vector.tensor_scalar_max(y1s1[:], y1s1[:], 0.0)
                y1s2 = fp.tile([64, DIM], F32)
                nc.vector.tensor_add(y1s2[:], y1b[:], mb1_sb[0:64, :])
                nc.vector.tensor_scalar_max(y1s2[:], y1s2[:], 0.0)
                y1T = fp.tile([128, c_.KD, c_.SEGP], F32)
                for kc in range(c_.KD):
                    pt = psf.tile([128, 128], F32, tag="ptr")
                    nc.tensor.transpose(pt[:], y1s1[:, kc * 128 : (kc + 1) * 128], ident_sb[:])
                    nc.vector.tensor_copy(y1T[:, kc, 0:128], pt[:])
                    pt2 = psf.tile([128, 64], F32, tag="ptr2", bufs=1)
                    nc.tensor.transpose(
                        pt2[:], y1s2[:, kc * 128 : (kc + 1) * 128], ident_sb[0:64, 0:64]
                    )
                    nc.vector.tensor_copy(y1T[:, kc, 128 : c_.SEGP], pt2[:])
                mw2_sb = fp.tile([128, c_.KD, c_.C], F32)
                nc.sync.dma_start(mw2_sb[:], mw2_d.ap().rearrange("(k p) d -> p k d", p=128))
                mb2_sb = fp.tile([128, c_.C], F32)
                nc.sync.dma_start(mb2_sb[:], mb2_d[:, :])
                la = psf.tile([128, c_.C], F32, tag="la", bufs=1)
                lb = psf.tile([64, c_.C], F32, tag="lb", bufs=1)
                for kc in range(c_.KD):
                    s0, s1 = (kc == 0), (kc == c_.KD - 1)
                    nc.tensor.matmul(la[:], y1T[:, kc, 0:128], mw2_sb[:, kc, :], start=s0, stop=s1)
                    nc.tensor.matmul(lb[:], y1T[:, kc, 128:192], mw2_sb[:, kc, :], start=s0, stop=s1)
                outs = []
                for ps_, bias_, np_ in ((la[:], mb2_sb[:], 128), (lb[:], mb2_sb[0:64, :], 64)):
                    lg = fp.tile([np_, c_.C], F32, tag=f"lg{np_}", name=f"lg{np_}")
                    nc.vector.tensor_add(lg[:], ps_, bias_)
                    mx = fp.tile([np_, 1], F32, tag=f"mx{np_}")
                    nc.vector.tensor_reduce(mx[:], lg[:], AX.X, ALU.max)
                    tt = fp.tile([np_, c_.C], F32, tag=f"tt{np_}")
                    nc.vector.tensor_scalar(tt[:], lg[:], mx[:], None, ALU.subtract)
                    ex = fp.tile([np_, c_.C], F32, tag=f"ex{np_}")
                    nc.scalar.activation(ex[:], tt[:], AF.Exp)
                    sm = fp.tile([np_, 1], F32, tag=f"sm{np_}")
                    nc.vector.tensor_reduce(sm[:], ex[:], AX.X, ALU.add)
                    ln = fp.tile([np_, 1], F32, tag=f"ln{np_}")
                    nc.scalar.activation(ln[:], sm[:], AF.Ln)
                    lp = fp.tile([np_, c_.C], F32, tag=f"lp{np_}")
                    nc.vector.tensor_scalar(lp[:], tt[:], ln[:], None, ALU.subtract)
                    outs.append(lp)
                nc.sync.dma_start(out_d[0:128, :], outs[0][:])
                nc.sync.dma_start(out_d[128 : c_.NORIG, :], outs[1][0 : c_.NORIG - 128, :])

    nc.compile()
    return nc


# --------------------------------------------------------------------------
# entry point
# --------------------------------------------------------------------------

def kernel(**inputs):
    meta, in_maps = prep(inputs, DEFAULT_CFG)
    nc = build(meta, DEFAULT_CFG)
    res = bass_utils.run_bass_kernel_spmd(nc, in_maps, core_ids=list(range(NCORES)))
    out = np.asarray(res.results[0]["out"], np.float32)
    return out


# revision 32
# speedup vs baseline: 1.3702x; 1.2709x over previous
"""Trainium2 Bass kernel for a 3-relation GCN (gnn_message_passing).

Strategy (8 NeuronCores, SPMD):
  - Node dim (20000 -> padded 20480) sharded 2560/core.
  - Layer GCN matmuls computed locally; per-relation [2560,256] fp16 shards
    AllGathered to a full node-feature table in DRAM.
  - Edge aggregation: edges partitioned by destination (host side), sorted,
    chunked into 128-edge groups per 64-dst window.  h rows are fetched with
    gpsimd dma_gather; a host-built one-hot scatter matrix S (carrying the
    symmetric GCN norms + bias edges) turns the scatter-add into PE matmuls
    accumulating in PSUM.
  - MLPs run from DMA-transposed (xbar) activations so the contraction dim
    lands on partitions.
  - Segment-mean pooling is a one-hot (1/cnt) matmul; the [192,768] fp32
    partials are AllReduced; the tiny final MLP + log_softmax is replicated.
All device math fp16 with fp32 PSUM accumulation; final stage fp32.
"""

import sys

import numpy as np

sys.path.insert(0, "/opt/trn_rl_repo")

import concourse.bacc as bacc  # noqa: E402
import concourse.bass as bass  # noqa: E402
import concourse.mybir as mybir  # noqa: E402
import concourse.tile as tile  # noqa: E402
from concourse import bass_utils  # noqa: E402

F16 = mybir.dt.float16
F32 = mybir.dt.float32
I16 = mybir.dt.int16
AF = mybir.ActivationFunctionType
ALU = mybir.AluOpType
AX = mybir.AxisListType

NCORES = 8


class Cfg:
    def __init__(self, n=20000, e=320000, f_in=5189, dim=256, norig=183, ncls=7,
                 npad=20480, fpad=5248, winw=64, nsup=10):
        self.N = n
        self.E = e
        self.F_IN = f_in
        self.DIM = dim
        self.NORIG = norig
        self.C = ncls
        self.NPAD = npad
        self.SH = npad // NCORES          # nodes per core
        self.NB = self.SH // 128          # 128-dst blocks per core
        self.WINW = winw                  # dst window width for scatter matmuls
        self.NWIN = self.SH // winw
        self.WPB = 128 // winw            # windows per block
        self.FPAD = fpad
        self.KC1 = fpad // 128
        self.NSUP = nsup                  # x stripes
        self.SUPW = self.SH // nsup
        self.HROWS = npad
        self.SEGP = 192                   # padded NORIG
        self.KCAT = 3 * dim // 128        # 6
        self.KD = dim // 128              # 2
        assert self.SH % 128 == 0 and self.SUPW % 128 == 0 and fpad % 128 == 0


DEFAULT_CFG = Cfg()


# --------------------------------------------------------------------------
# host-side preprocessing
# --------------------------------------------------------------------------

def _wrap_idx(idx, cfg):
    """dma_gather index layout: pos i -> partition i%16, col i//16; the 16-row
    block replicated across the 8 GPSIMD cores (128 partitions)."""
    tg8 = idx.shape[0] // 16
    w = idx.astype(np.int16).reshape(tg8, 16).T            # [16, tg*8]
    return np.ascontiguousarray(np.tile(w, (8, 1)))        # [128, tg*8]


def prep(inputs, cfg=DEFAULT_CFG):
    c_ = cfg
    f16 = np.float16
    x = np.asarray(inputs["x"], np.float32)
    w1 = np.asarray(inputs["w1"], np.float32)
    b1 = np.asarray(inputs["b1"], np.float32)
    w2 = np.asarray(inputs["w2"], np.float32)
    b2 = np.asarray(inputs["b2"], np.float32)
    eis = [np.asarray(inputs[f"edge_index_{k}"], np.int64) for k in (1, 2, 3)]
    idx1 = np.asarray(inputs["index_1"], np.int64)
    idx3 = np.asarray(inputs["index_3"], np.int64)

    # ---- per-relation edge prep: self loops, degrees, norms, dst partition
    Gw = np.zeros((3, c_.NWIN), np.int64)   # groups per (relative) window, max over cores
    core_edges = [[None] * 3 for _ in range(NCORES)]
    loop = np.arange(c_.N)
    dinv2_all = np.zeros((3, c_.NPAD), np.float32)  # self-loop coeff per node
    for r in range(3):
        src, dst = eis[r][0], eis[r][1]
        deg = np.bincount(dst, minlength=c_.N).astype(np.float64) + 1.0  # + self loop
        dinv = 1.0 / np.sqrt(deg)
        dinv2_all[r, : c_.N] = (dinv * dinv).astype(np.float32)
        norm = (dinv[src] * dinv[dst]).astype(np.float32)
        for c in range(NCORES):
            lo = c * c_.SH
            m = (dst >= lo) & (dst < lo + c_.SH)
            es, ed, en = src[m], dst[m] - lo, norm[m]
            o = np.argsort(ed, kind="stable")
            es, ed, en = es[o], ed[o], en[o]
            core_edges[c][r] = (es, ed, en)
            cnts = np.bincount(ed // c_.WINW, minlength=c_.NWIN)
            Gw[r] = np.maximum(Gw[r], (cnts + 127) // 128)
    Gw = np.maximum(Gw, 1)  # every window needs >=1 group so its PSUM is written
    TG = Gw.sum(axis=1)

    # ---- pooling one-hot (1/cnt) matrices, global counts
    cnt1 = np.bincount(idx1, minlength=c_.NORIG).astype(np.float64)
    cnt3 = np.bincount(idx3, minlength=c_.NORIG).astype(np.float64)
    r1 = (1.0 / np.maximum(cnt1, 1.0)).astype(np.float32)
    r3 = (1.0 / np.maximum(cnt3, 1.0)).astype(np.float32)
    P1 = np.zeros((NCORES, 128, c_.NB, c_.SEGP), f16)
    P3 = np.zeros((NCORES, 128, c_.NB, c_.SEGP), f16)
    nn = np.arange(c_.N)
    cc, ll = nn // c_.SH, nn % c_.SH
    P1[cc, ll % 128, ll // 128, idx1] = r1[idx1].astype(f16)
    P3[cc, ll % 128, ll // 128, idx3] = r3[idx3].astype(f16)

    # ---- shared (replicated) tensors
    w1h = np.zeros((3, c_.FPAD, c_.DIM), f16)
    w1h[:, : c_.F_IN] = w1.astype(f16)
    shared = {
        "w1h": w1h,
        "w2h": w2.astype(f16),
        "b1bc": np.ascontiguousarray(np.broadcast_to(b1[:, None, :], (3, 128, c_.DIM)).astype(np.float32)),
        "b2bc": np.ascontiguousarray(np.broadcast_to(b2[:, None, :], (3, 128, c_.DIM)).astype(np.float32)),
        "m1w1h": np.asarray(inputs["m1w1"], np.float32).astype(f16),
        "m1w2h": np.asarray(inputs["m1w2"], np.float32).astype(f16),
        "m2w1h": np.asarray(inputs["m2w1"], np.float32).astype(f16),
        "m2w2h": np.asarray(inputs["m2w2"], np.float32).astype(f16),
        "m1b1bc": np.tile(np.asarray(inputs["m1b1"], np.float32), (128, 1)),
        "m1b2bc": np.tile(np.asarray(inputs["m1b2"], np.float32), (128, 1)),
        "m2b1bc": np.tile(np.asarray(inputs["m2b1"], np.float32), (128, 1)),
        "m2b2bc": np.tile(np.asarray(inputs["m2b2"], np.float32), (128, 1)),
        "mw1f": np.asarray(inputs["mw1"], np.float32),
        "mw2f": np.asarray(inputs["mw2"], np.float32),
        "mb1bc": np.tile(np.asarray(inputs["mb1"], np.float32), (128, 1)),
        "mb2bc": np.tile(np.asarray(inputs["mb2"], np.float32), (128, 1)),
        "ident": np.eye(128, dtype=np.float32),
    }

    # ---- per-core tensors
    in_maps = []
    for c in range(NCORES):
        d = dict(shared)
        lo = c * c_.SH
        nreal = int(min(max(c_.N - lo, 0), c_.SH))
        xs = np.zeros((c_.SH, c_.FPAD), f16)
        xs[:nreal, : c_.F_IN] = x[lo : lo + nreal].astype(f16)
        xT = np.ascontiguousarray(
            xs.T.reshape(c_.FPAD, c_.NSUP, c_.SUPW).transpose(1, 0, 2)
        )
        d["xT"] = xT
        for r in range(3):
            es, ed, en = core_edges[c][r]
            win = ed // c_.WINW
            tg = int(TG[r])
            sp = np.zeros(tg * 128, np.int64)
            dl = np.zeros(tg * 128, np.int64)
            nm = np.zeros(tg * 128, np.float32)
            st = np.searchsorted(win, np.arange(c_.NWIN), side="left")
            en_ = np.searchsorted(win, np.arange(c_.NWIN), side="right")
            g0 = 0
            for w in range(c_.NWIN):
                cnt = int(en_[w] - st[w])
                sl = slice(g0 * 128, g0 * 128 + cnt)
                sp[sl] = es[st[w] : en_[w]]
                dl[sl] = ed[st[w] : en_[w]] - w * c_.WINW
                nm[sl] = en[st[w] : en_[w]]
                g0 += int(Gw[r][w])
            S = np.zeros((tg * 128, c_.WINW), f16)
            S[np.arange(tg * 128), dl] = nm.astype(f16)  # pad rows write 0.0
            d[f"S_{r}"] = np.ascontiguousarray(
                S.reshape(tg, 128, c_.WINW).transpose(1, 0, 2).reshape(128, tg * c_.WINW)
            )
            d[f"gidx_{r}"] = _wrap_idx(sp, c_)
            d[f"dinv2_{r}"] = np.ascontiguousarray(
                dinv2_all[r, lo : lo + c_.SH].reshape(c_.NB, 128).T
            )
        d["P1"] = np.ascontiguousarray(P1[c])
        d["P3"] = np.ascontiguousarray(P3[c])
        in_maps.append(d)

    meta = {"Gw": Gw.astype(int), "TG": [int(t) for t in TG]}
    return meta, in_maps


# --------------------------------------------------------------------------
# device program
# --------------------------------------------------------------------------

def build(meta, cfg=DEFAULT_CFG, enable_asserts=False):
    c_ = cfg
    Gw, TG = meta["Gw"], meta["TG"]
    nc = bacc.Bacc(
        "TRN2",
        target_bir_lowering=False,
        debug=False,
        enable_asserts=enable_asserts,
        num_devices=NCORES,
        num_swdge_queues=4,
    )

    # ---------------- DRAM I/O
    xT_d = nc.dram_tensor("xT", [c_.NSUP, c_.FPAD, c_.SUPW], F16, kind="ExternalInput")
    w1_d = nc.dram_tensor("w1h", [3, c_.FPAD, c_.DIM], F16, kind="ExternalInput")
    w2_d = nc.dram_tensor("w2h", [3, c_.DIM, c_.DIM], F16, kind="ExternalInput")
    b1_d = nc.dram_tensor("b1bc", [3, 128, c_.DIM], F32, kind="ExternalInput")
    b2_d = nc.dram_tensor("b2bc", [3, 128, c_.DIM], F32, kind="ExternalInput")
    dinv2_d = [
        nc.dram_tensor(f"dinv2_{r}", [128, c_.NB], F32, kind="ExternalInput")
        for r in range(3)
    ]
    m1w1_d = nc.dram_tensor("m1w1h", [3 * c_.DIM, c_.DIM], F16, kind="ExternalInput")
    m1w2_d = nc.dram_tensor("m1w2h", [c_.DIM, c_.DIM], F16, kind="ExternalInput")
    m2w1_d = nc.dram_tensor("m2w1h", [3 * c_.DIM, c_.DIM], F16, kind="ExternalInput")
    m2w2_d = nc.dram_tensor("m2w2h", [c_.DIM, c_.DIM], F16, kind="ExternalInput")
    m1b1_d = nc.dram_tensor("m1b1bc", [128, c_.DIM], F32, kind="ExternalInput")
    m1b2_d = nc.dram_tensor("m1b2bc", [128, c_.DIM], F32, kind="ExternalInput")
    m2b1_d = nc.dram_tensor("m2b1bc", [128, c_.DIM], F32, kind="ExternalInput")
    m2b2_d = nc.dram_tensor("m2b2bc", [128, c_.DIM], F32, kind="ExternalInput")
    mw1_d = nc.dram_tensor("mw1f", [3 * c_.DIM, c_.DIM], F32, kind="ExternalInput")
    mw2_d = nc.dram_tensor("mw2f", [c_.DIM, c_.C], F32, kind="ExternalInput")
    mb1_d = nc.dram_tensor("mb1bc", [128, c_.DIM], F32, kind="ExternalInput")
    mb2_d = nc.dram_tensor("mb2bc", [128, c_.C], F32, kind="ExternalInput")
    ident_d = nc.dram_tensor("ident", [128, 128], F32, kind="ExternalInput")
    S_d = [
        nc.dram_tensor(f"S_{r}", [128, TG[r] * c_.WINW], F16, kind="ExternalInput")
        for r in range(3)
    ]
    gidx_d = [
        nc.dram_tensor(f"gidx_{r}", [128, TG[r] * 8], I16, kind="ExternalInput")
        for r in range(3)
    ]
    P1_d = nc.dram_tensor("P1", [128, c_.NB, c_.SEGP], F16, kind="ExternalInput")
    P3_d = nc.dram_tensor("P3", [128, c_.NB, c_.SEGP], F16, kind="ExternalInput")
    out_d = nc.dram_tensor("out", [c_.NORIG, c_.C], F32, kind="ExternalOutput")

    groups_all = list(range(NCORES))
    DIM = c_.DIM

    with tile.TileContext(nc) as tc:
        with tc.tile_pool(name="dram", bufs=1, space="DRAM") as dpool, \
             tc.tile_pool(name="consts", bufs=1) as cpool:
            # DRAM intermediates
            h_loc = [[dpool.tile([c_.SH, DIM], F16, name=f"hloc{l}_{r}")
                      for r in range(3)] for l in range(2)]
            h_full = [[dpool.tile([c_.HROWS, DIM], F16, name=f"hfull{l}_{r}")
                       for r in range(3)] for l in range(2)]
            hcat_dram = [dpool.tile([c_.SH, 3 * DIM], F16, name=f"hcat{l}")
                         for l in range(2)]
            z1_dram = [dpool.tile([c_.SH, DIM], F16, name=f"z1d{l}") for l in range(2)]
            h2_dram = dpool.tile([c_.SH, DIM], F16, name="h2d")
            xcat_dram = dpool.tile([c_.SEGP, 3 * DIM], F32, name="xcatd")
            xcat_red = dpool.tile([c_.SEGP, 3 * DIM], F32, name="xcatr")

            # persistent small consts
            bbc_sb = cpool.tile([128, 2, 3, DIM], F32)     # bias bcast, [layer][rel]
            nc.sync.dma_start(bbc_sb[:, 0, :, :], b1_d.ap().rearrange("r p d -> p r d"))
            nc.sync.dma_start(bbc_sb[:, 1, :, :], b2_d.ap().rearrange("r p d -> p r d"))
            dinv2_sb = cpool.tile([128, 3, c_.NB], F32)
            for r in range(3):
                nc.sync.dma_start(dinv2_sb[:, r, :], dinv2_d[r][:, :])
            idx_sb = []
            for r in range(3):
                t = cpool.tile([128, TG[r] * 8], I16, name=f"idx{r}")
                nc.sync.dma_start(t[:], gidx_d[r][:, :])
                idx_sb.append(t)

            # ----------------------------------------------------------------
            def allgather(l, r):
                nc.gpsimd.collective_compute(
                    "AllGather",
                    ALU.bypass,
                    replica_groups=[groups_all],
                    ins=[h_loc[l][r][:, :]],
                    outs=[h_full[l][r][0 : c_.NPAD, :]],
                )

            def aggregate(l, r, gpool, spool, psp, stp):
                """scatter-add via one-hot matmuls; writes relu result to hcat."""
                gb = 0
                for b in range(c_.NB):
                    qn = b % 4
                    nws = [int(Gw[r][c_.WPB * b + k]) for k in range(c_.WPB)]
                    Gb = sum(nws)
                    gt = gpool.tile([128, Gb, DIM], F16, tag="gath")
                    nc.gpsimd.dma_gather(
                        gt[:],
                        h_full[l][r][:, :],
                        idx_sb[r][:, gb * 8 : (gb + Gb) * 8],
                        Gb * 128,
                        Gb * 128,
                        DIM,
                        # single_packet caps at 64 descs/engine = 1024 idxs
                        single_packet=(Gb * 128 <= 1024),
                        queue_num=qn,
                    )
                    s_sb = spool.tile([128, Gb, c_.WINW], F16, tag="sgt")
                    nc.sync.dma_start(
                        s_sb[:], S_d[r][:, gb * c_.WINW : (gb + Gb) * c_.WINW]
                    )
                    ps = psp.tile([128, DIM], F32, tag="psagg")
                    g = 0
                    for k in range(c_.WPB):
                        w0 = k * c_.WINW
                        for j in range(nws[k]):
                            nc.tensor.matmul(
                                ps[w0 : w0 + c_.WINW, :],
                                s_sb[:, g, :],
                                gt[:, g, :],
                                start=(j == 0),
                                stop=(j == nws[k] - 1),
                            )
                            g += 1
                    # self-loop (dinv^2 * h_local) + bias + relu, fused on evict
                    hl = stp.tile([128, DIM], F16, tag="hlocst")
                    nc.sync.dma_start(hl[:], h_loc[l][r][b * 128 : (b + 1) * 128, :])
                    t1 = stp.tile([128, DIM], F32, tag="aggt1")
                    nc.vector.tensor_scalar(
                        t1[:], hl[:], dinv2_sb[:, r, b : b + 1], None, ALU.mult
                    )
                    nc.vector.tensor_add(t1[:], t1[:], ps[:])
                    nc.vector.tensor_add(t1[:], t1[:], bbc_sb[:, l, r, :])
                    st = stp.tile([128, DIM], F16, tag="aggst")
                    nc.vector.tensor_scalar_max(st[:], t1[:], 0.0)  # relu + cast
                    nc.sync.dma_start(
                        hcat_dram[l][b * 128 : (b + 1) * 128, r * DIM : (r + 1) * DIM],
                        st[:],
                    )
                    gb += Gb

            # ================================================================
            # Layer 1: x @ W1_r  (lhsT = xT stripes)
            # ================================================================
            with tc.tile_pool(name="w1p", bufs=1) as w1p, \
                 tc.tile_pool(name="xsp", bufs=2) as xsp, \
                 tc.tile_pool(name="psa", bufs=2, space="PSUM") as psa, \
                 tc.tile_pool(name="hst", bufs=6) as hstp:
                w1_sb = w1p.tile([128, 3 * c_.KC1, DIM], F16)
                for r in range(3):
                    nc.sync.dma_start(
                        w1_sb[:, r * c_.KC1 : (r + 1) * c_.KC1, :],
                        w1_d[r].rearrange("(k p) d -> p k d", p=128),
                    )
                ntps = c_.SUPW // 128
                for ns in range(c_.NSUP):
                    xs_sb = xsp.tile([128, c_.KC1, c_.SUPW], F16, tag="xs")
                    nc.sync.dma_start(
                        xs_sb[:], xT_d[ns].rearrange("(k p) w -> p k w", p=128)
                    )
                    for ntl in range(ntps):
                        nt = ns * ntps + ntl
                        pss = [psa.tile([128, DIM], F32, tag=f"psa{r}", name=f"psa{r}")
                               for r in range(3)]
                        for kc in range(c_.KC1):
                            lhsT = xs_sb[:, kc, ntl * 128 : (ntl + 1) * 128]
                            for r in range(3):
                                nc.tensor.matmul(
                                    pss[r][:],
                                    lhsT,
                                    w1_sb[:, r * c_.KC1 + kc, :],
                                    start=(kc == 0),
                                    stop=(kc == c_.KC1 - 1),
                                )
                        for r in range(3):
                            st = hstp.tile([128, DIM], F16, tag="hstage")
                            nc.vector.tensor_copy(st[:], pss[r][:])
                            nc.sync.dma_start(
                                h_loc[0][r][nt * 128 : (nt + 1) * 128, :], st[:]
                            )

            # ================================================================
            # per-layer: allgather -> aggregate -> MLP
            # ================================================================
            for l in range(2):
                for r in range(3):
                    allgather(l, r)
                with tc.tile_pool(name=f"gp{l}", bufs=8) as gpool, \
                     tc.tile_pool(name=f"sp{l}", bufs=8) as spool, \
                     tc.tile_pool(name=f"pc{l}", bufs=4, space="PSUM") as psp, \
                     tc.tile_pool(name=f"st{l}", bufs=6) as stp:
                    for r in range(3):
                        aggregate(l, r, gpool, spool, psp, stp)

                # MLP: hcat -> z1 (relu) -> h2
                w1name = m1w1_d if l == 0 else m2w1_d
                w2name = m1w2_d if l == 0 else m2w2_d
                bb1 = m1b1_d if l == 0 else m2b1_d
                bb2 = m1b2_d if l == 0 else m2b2_d
                with tc.tile_pool(name=f"mlp{l}", bufs=1) as mp, \
                     tc.tile_pool(name=f"psm{l}", bufs=2, space="PSUM") as psm, \
                     tc.tile_pool(name=f"mst{l}", bufs=4) as mst:
                    hcatT = mp.tile([128, c_.KCAT, c_.SH], F16)
                    for kt in range(c_.KCAT):
                        nc.sync.dma_start_transpose(
                            hcatT[:, kt, :], hcat_dram[l][:, kt * 128 : (kt + 1) * 128]
                        )
                    mw1_sb = mp.tile([128, c_.KCAT, DIM], F16)
                    nc.sync.dma_start(mw1_sb[:], w1name.ap().rearrange("(k p) d -> p k d", p=128))
                    mw2_sb = mp.tile([128, c_.KD, DIM], F16)
                    nc.sync.dma_start(mw2_sb[:], w2name.ap().rearrange("(k p) d -> p k d", p=128))
                    bb1_sb = mp.tile([128, DIM], F32)
                    nc.sync.dma_start(bb1_sb[:], bb1[:, :])
                    bb2_sb = mp.tile([128, DIM], F32)
                    nc.sync.dma_start(bb2_sb[:], bb2[:, :])

                    for nt in range(c_.NB):
                        ps = psm.tile([128, DIM], F32, tag="psz1")
                        for kc in range(c_.KCAT):
                            nc.tensor.matmul(
                                ps[:],
                                hcatT[:, kc, nt * 128 : (nt + 1) * 128],
                                mw1_sb[:, kc, :],
                                start=(kc == 0),
                                stop=(kc == c_.KCAT - 1),
                            )
                        tmp = mst.tile([128, DIM], F32, tag="ztmp")
                        nc.vector.tensor_add(tmp[:], ps[:], bb1_sb[:])
                        z1st = mst.tile([128, DIM], F16, tag="z1st")
                        nc.vector.tensor_scalar_max(z1st[:], tmp[:], 0.0)
                        nc.sync.dma_start(
                            z1_dram[l][nt * 128 : (nt + 1) * 128, :], z1st[:]
                        )
                    z1T = mp.tile([128, c_.KD, c_.SH], F16)
                    for kt in range(c_.KD):
                        nc.sync.dma_start_transpose(
                            z1T[:, kt, :], z1_dram[l][:, kt * 128 : (kt + 1) * 128]
                        )
                    if l == 0:
                        for nt in range(c_.NB):
                            ps = psm.tile([128, DIM], F32, tag="psz2")
                            for kc in range(c_.KD):
                                nc.tensor.matmul(
                                    ps[:],
                                    z1T[:, kc, nt * 128 : (nt + 1) * 128],
                                    mw2_sb[:, kc, :],
                                    start=(kc == 0),
                                    stop=(kc == c_.KD - 1),
                                )
                            h2st = mst.tile([128, DIM], F16, tag="h2st")
                            nc.vector.tensor_add(h2st[:], ps[:], bb2_sb[:])
                            nc.sync.dma_start(
                                h2_dram[nt * 128 : (nt + 1) * 128, :], h2st[:]
                            )
                        # layer-2 GCN matmuls: h2 @ W2_r
                        h2T = mp.tile([128, c_.KD, c_.SH], F16)
                        for kt in range(c_.KD):
                            nc.sync.dma_start_transpose(
                                h2T[:, kt, :], h2_dram[:, kt * 128 : (kt + 1) * 128]
                            )
                        w2_sb = mp.tile([128, 3 * c_.KD, DIM], F16)
                        for r in range(3):
                            nc.sync.dma_start(
                                w2_sb[:, r * c_.KD : (r + 1) * c_.KD, :],
                                w2_d[r].rearrange("(k p) d -> p k d", p=128),
                            )
                        for nt in range(c_.NB):
                            pss = [psm.tile([128, DIM], F32, tag=f"psg{r}",
                                            name=f"psg{r}", bufs=1) for r in range(3)]
                            for kc in range(c_.KD):
                                lhsT = h2T[:, kc, nt * 128 : (nt + 1) * 128]
                                for r in range(3):
                                    nc.tensor.matmul(
                                        pss[r][:],
                                        lhsT,
                                        w2_sb[:, r * c_.KD + kc, :],
                                        start=(kc == 0),
                                        stop=(kc == c_.KD - 1),
                                    )
                            for r in range(3):
                                st = mst.tile([128, DIM], F16, tag="hstage2")
                                nc.vector.tensor_copy(st[:], pss[r][:])
                                nc.sync.dma_start(
                                    h_loc[1][r][nt * 128 : (nt + 1) * 128, :], st[:]
                                )
                    else:
                        # final node features h3 (kept in SBUF)
                        h3_sb = cpool.tile([128, c_.NB, DIM], F16)
                        for nt in range(c_.NB):
                            ps = psm.tile([128, DIM], F32, tag="psz2")
                            for kc in range(c_.KD):
                                nc.tensor.matmul(
                                    ps[:],
                                    z1T[:, kc, nt * 128 : (nt + 1) * 128],
                                    mw2_sb[:, kc, :],
                                    start=(kc == 0),
                                    stop=(kc == c_.KD - 1),
                                )
                            nc.vector.tensor_add(h3_sb[:, nt, :], ps[:], bb2_sb[:])

            # ================================================================
            # segment-mean pooling + AllReduce
            # ================================================================
            with tc.tile_pool(name="segp", bufs=1) as sgp, \
                 tc.tile_pool(name="psx", bufs=1, space="PSUM") as psx:
                P1_sb = sgp.tile([128, c_.NB, c_.SEGP], F16)
                nc.sync.dma_start(P1_sb[:], P1_d[:, :, :])
                P3_sb = sgp.tile([128, c_.NB, c_.SEGP], F16)
                nc.sync.dma_start(P3_sb[:], P3_d[:, :, :])
                x1a = psx.tile([128, DIM], F32, tag="x1a", bufs=1)
                x1b = psx.tile([64, DIM], F32, tag="x1b", bufs=1)
                x3a = psx.tile([128, DIM], F32, tag="x3a", bufs=1)
                x3b = psx.tile([64, DIM], F32, tag="x3b", bufs=1)
                for nt in range(c_.NB):
                    rhs = h3_sb[:, nt, :]
                    s0, s1 = (nt == 0), (nt == c_.NB - 1)
                    nc.tensor.matmul(x1a[:], P1_sb[:, nt, 0:128], rhs, start=s0, stop=s1)
                    nc.tensor.matmul(x1b[:], P1_sb[:, nt, 128:192], rhs, start=s0, stop=s1)
                    nc.tensor.matmul(x3a[:], P3_sb[:, nt, 0:128], rhs, start=s0, stop=s1)
                    nc.tensor.matmul(x3b[:], P3_sb[:, nt, 128:192], rhs, start=s0, stop=s1)
                xc1 = sgp.tile([128, 3 * DIM], F32)
                xc2 = sgp.tile([64, 3 * DIM], F32)
                nc.vector.tensor_copy(xc1[:, 0:DIM], x1a[:])
                nc.vector.tensor_copy(xc1[:, DIM : 2 * DIM], x3a[:])
                nc.vector.tensor_copy(xc1[:, 2 * DIM : 3 * DIM], x3a[:])
                nc.vector.tensor_copy(xc2[:, 0:DIM], x1b[:])
                nc.vector.tensor_copy(xc2[:, DIM : 2 * DIM], x3b[:])
                nc.vector.tensor_copy(xc2[:, 2 * DIM : 3 * DIM], x3b[:])
                nc.sync.dma_start(xcat_dram[0:128, :], xc1[:])
                nc.sync.dma_start(xcat_dram[128 : c_.SEGP, :], xc2[:])
            nc.gpsimd.collective_compute(
                "AllReduce",
                ALU.add,
                replica_groups=[groups_all],
                ins=[xcat_dram[:, :]],
                outs=[xcat_red[:, :]],
            )

            # ================================================================
            # final MLP (fp32) + log_softmax, replicated
            # ================================================================
            with tc.tile_pool(name="fin", bufs=1) as fp, \
                 tc.tile_pool(name="psf", bufs=2, space="PSUM") as psf:
                ident_sb = fp.tile([128, 128], F32)
                nc.sync.dma_start(ident_sb[:], ident_d[:, :])
                xr1 = fp.tile([128, 3 * DIM], F32)
                nc.sync.dma_start(xr1[:], xcat_red[0:128, :])
                xr2 = fp.tile([64, 3 * DIM], F32)
                nc.sync.dma_start(xr2[:], xcat_red[128 : c_.SEGP, :])
                xcT = fp.tile([128, c_.KCAT, c_.SEGP], F32)
                for kc in range(c_.KCAT):
                    pt = psf.tile([128, 128], F32, tag="ptr")
                    nc.tensor.transpose(pt[:], xr1[:, kc * 128 : (kc + 1) * 128], ident_sb[:])
                    nc.vector.tensor_copy(xcT[:, kc, 0:128], pt[:])
                    pt2 = psf.tile([128, 64], F32, tag="ptr2", bufs=1)
                    nc.tensor.transpose(
                        pt2[:], xr2[:, kc * 128 : (kc + 1) * 128], ident_sb[0:64, 0:64]
                    )
                    nc.vector.tensor_copy(xcT[:, kc, 128 : c_.SEGP], pt2[:])
                mw1_sb = fp.tile([128, c_.KCAT, DIM], F32)
                nc.sync.dma_start(mw1_sb[:], mw1_d.ap().rearrange("(k p) d -> p k d", p=128))
                mb1_sb = fp.tile([128, DIM], F32)
                nc.sync.dma_start(mb1_sb[:], mb1_d[:, :])
                y1a = psf.tile([128, DIM], F32, tag="y1a", bufs=1)
                y1b = psf.tile([64, DIM], F32, tag="y1b", bufs=1)
                for kc in range(c_.KCAT):
                    s0, s1 = (kc == 0), (kc == c_.KCAT - 1)
                    nc.tensor.matmul(y1a[:], xcT[:, kc, 0:128], mw1_sb[:, kc, :], start=s0, stop=s1)
                    nc.tensor.matmul(y1b[:], xcT[:, kc, 128:192], mw1_sb[:, kc, :], start=s0, stop=s1)
                y1s1 = fp.tile([128, DIM], F32)
                nc.vector.tensor_add(y1s1[:], y1a[:], mb1_sb[:])
                nc.vector.tensor_scalar_max(y1s1[:], y1s1[:], 0.0)
                y1s2 = fp.tile([64, DIM], F32)
                nc.vector.tensor_add(y1s2[:], y1b[:], mb1_sb[0:64, :])
                nc.vector.tensor_scalar_max(y1s2[:], y1s2[:], 0.0)
                y1T = fp.tile([128, c_.KD, c_.SEGP], F32)
                for kc in range(c_.KD):
                    pt = psf.tile([128, 128], F32, tag="ptr")
                    nc.tensor.transpose(pt[:], y1s1[:, kc * 128 : (kc + 1) * 128], ident_sb[:])
                    nc.vector.tensor_copy(y1T[:, kc, 0:128], pt[:])
                    pt2 = psf.tile([128, 64], F32, tag="ptr2", bufs=1)
                    nc.tensor.transpose(
                        pt2[:], y1s2[:, kc * 128 : (kc + 1) * 128], ident_sb[0:64, 0:64]
                    )
                    nc.vector.tensor_copy(y1T[:, kc, 128 : c_.SEGP], pt2[:])
                mw2_sb = fp.tile([128, c_.KD, c_.C], F32)
                nc.sync.dma_start(mw2_sb[:], mw2_d.ap().rearrange("(k p) d -> p k d", p=128))
                mb2_sb = fp.tile([128, c_.C], F32)
                nc.sync.dma_start(mb2_sb[:], mb2_d[:, :])
                la = psf.tile([128, c_.C], F32, tag="la", bufs=1)
                lb = psf.tile([64, c_.C], F32, tag="lb", bufs=1)
                for kc in range(c_.KD):
                    s0, s1 = (kc == 0), (kc == c_.KD - 1)
                    nc.tensor.matmul(la[:], y1T[:, kc, 0:128], mw2_sb[:, kc, :], start=s0, stop=s1)
                    nc.tensor.matmul(lb[:], y1T[:, kc, 128:192], mw2_sb[:, kc, :], start=s0, stop=s1)
                outs = []
                for ps_, bias_, np_ in ((la[:], mb2_sb[:], 128), (lb[:], mb2_sb[0:64, :], 64)):
                    lg = fp.tile([np_, c_.C], F32, tag=f"lg{np_}", name=f"lg{np_}")
                    nc.vector.tensor_add(lg[:], ps_, bias_)
                    mx = fp.tile([np_, 1], F32, tag=f"mx{np_}")
                    nc.vector.tensor_reduce(mx[:], lg[:], AX.X, ALU.max)
                    tt = fp.tile([np_, c_.C], F32, tag=f"tt{np_}")
                    nc.vector.tensor_scalar(tt[:], lg[:], mx[:], None, ALU.subtract)
                    ex = fp.tile([np_, c_.C], F32, tag=f"ex{np_}")
                    nc.scalar.activation(ex[:], tt[:], AF.Exp)
                    sm = fp.tile([np_, 1], F32, tag=f"sm{np_}")
                    nc.vector.tensor_reduce(sm[:], ex[:], AX.X, ALU.add)
                    ln = fp.tile([np_, 1], F32, tag=f"ln{np_}")
                    nc.scalar.activation(ln[:], sm[:], AF.Ln)
                    lp = fp.tile([np_, c_.C], F32, tag=f"lp{np_}")
                    nc.vector.tensor_scalar(lp[:], tt[:], ln[:], None, ALU.subtract)
                    outs.append(lp)
                nc.sync.dma_start(out_d[0:128, :], outs[0][:])
                nc.sync.dma_start(out_d[128 : c_.NORIG, :], outs[1][0 : c_.NORIG - 128, :])

    nc.compile()
    return nc


# --------------------------------------------------------------------------
# entry point
# --------------------------------------------------------------------------

def kernel(**inputs):
    meta, in_maps = prep(inputs, DEFAULT_CFG)
    nc = build(meta, DEFAULT_CFG)
    res = bass_utils.run_bass_kernel_spmd(nc, in_maps, core_ids=list(range(NCORES)))
    out = np.asarray(res.results[0]["out"], np.float32)
    return out
